# revision 31
# baseline (speedup 1.0000x reference)
"""Trainium2 Bass kernel for nn_AttnDecoder (B=8192, T=10, CH=H=512).

Math notes (verified against the jax reference in fp32 to ~3e-6):
  - The attention block is dead code: softmax over a size-1 axis == 1, so
    h1 == ht and attn1/2/3 never affect the output.
  - The LSTM hidden state d never feeds back into the gates (only the cell
    state c does, elementwise), so the only sequential part is
        c_t = sigmoid(f_t) * c_{t-1} + sigmoid(i_t) * tanh(g_t)
    a cheap elementwise recurrence over T=10.
  - o-gate is only needed at t = T-1.
  - fc2(fc1(z)) with no nonlinearity folds into a single vector:
        y = d . v[:H] + h9 . v[H:] + c0,   v = (fc2_w @ fc1_w)^T.

Sharding: batch-parallel over 8 cores (1024 rows each), weights replicated.

Implementation (v2, fp8 DoubleRow):
  - h is pre-cast to fp8-e4m3 and pre-transposed on the host into
    hT8[t, p, k*1024+b] = fp8(h[b, t, 128k+p]); whole-tensor rel-err budget
    allows it (fp8 pipeline sims at 8.3e-3 vs the 2e-2 gate).
  - Gate matmuls run in fp8 DoubleRow mode: each instruction contracts
    K=256 (two 128-channel regions, lhsT/rhs APs shaped [128, 2, n]),
    halving PE streaming time vs bf16.
  - Weights are pre-scaled by S=16 on the host (better fp8 mantissa use);
    the activation instruction compensates with scale=1/S for free.
  - y_t / bias enter via a K=2 bf16 matmul (rows [y;1] x [w_ih*S; b*S]),
    4-up tile_position-packed, accumulating into the same PSUM banks.
  - Cell state c is bf16 (DVE 2x mode); sim shows no accuracy change.
  - Everything is resident in SBUF up-front (~120KB/partition): no SWDGE,
    no DRAM staging, no transpose DMAs, no gpsimd.
  - Final: d = sigma(o)*tanh(c); y_d via fp32 PE dot with v[:H];
    y_h = h9 . v[H:] fused mult+reduce on DVE; partials summed on host.
"""

import numpy as np
import ml_dtypes

import concourse.bass as bass
import concourse.tile as tile
from concourse import bacc, mybir
from concourse.bass_utils import run_bass_kernel_spmd

BF16 = ml_dtypes.bfloat16
F8 = ml_dtypes.float8_e4m3

B, T, CH, H = 8192, 10, 512, 512
N_CORES = 8
B_LOC = B // N_CORES            # 1024 batch rows per core
P = 128
S = 16.0                        # fp8 weight pre-scale
Y8 = False                      # y/bias matmul operands in fp8 DR form

_compiled = {}


def build_nc(b_loc=B_LOC, bgrp=512, psum_bufs=2, work_bufs=2,
             use_ttr=False, c_bf16=True, use_dr=True, y8=Y8,
             pair_gates=False):
    NBG = b_loc // bgrp         # batch groups
    NJ = H // P                 # 4 hid chunks
    NQ = CH // (2 * P)          # 2 DoubleRow K-chunks (256 channels each)
    NBT = b_loc // P            # batch tiles for the h9 dot
    GW = NJ * bgrp              # big-tile width (one gate, all hid chunks)
    f32 = mybir.dt.float32
    bf16 = mybir.dt.bfloat16
    f8 = mybir.dt.float8e4
    AF = mybir.ActivationFunctionType
    ALU = mybir.AluOpType
    DR = mybir.MatmulPerfMode.DoubleRow

    nc = bacc.Bacc("TRN2", target_bir_lowering=False, debug=False,
                   num_devices=N_CORES)

    # hT8[t, p, k*b_loc + b] = fp8(h[b, t, 128k + p])
    hT_in = nc.dram_tensor("hT8", [T, P, NJ * b_loc], f8, kind="ExternalInput")
    h9_in = nc.dram_tensor("h9", [b_loc, CH], bf16, kind="ExternalInput")
    # per t: rhs rows [y_t ; ones] for the K=2 bias/y matmul
    if y8:
        yt_in = nc.dram_tensor("yt_aug", [1, 2, T * b_loc], f8,
                               kind="ExternalInput")
        wb8_in = nc.dram_tensor("wb8", [1, 2, 4 * H], f8,
                                kind="ExternalInput")
    else:
        yt_in = nc.dram_tensor("yt_aug", [2, T, b_loc], bf16,
                               kind="ExternalInput")
    # w8[q, p, i*2048 + m] = fp8(W_hh[m, 128*(2q+i) + p] * S)
    w8_in = nc.dram_tensor("w8", [NQ, P, 2 * 4 * H], f8, kind="ExternalInput")
    if not y8:
        # rows: [w_ih * S ; (b_ih + b_hh) * S]
        wb_in = nc.dram_tensor("wih_b", [2, 4 * H], bf16,
                               kind="ExternalInput")
    vd_in = nc.dram_tensor("v_d", [P, NJ], bf16, kind="ExternalInput")
    vh_in = nc.dram_tensor("v_h", [P, CH], bf16, kind="ExternalInput")
    outd = nc.dram_tensor("out_d", [b_loc], f32, kind="ExternalOutput")
    outh = nc.dram_tensor("out_h", [P, NBT], f32, kind="ExternalOutput")

    G_I, G_F, G_G, G_O = 0, 1, 2, 3     # gate blocks in the 2048 W columns
    INV = 1.0 / S

    with tile.TileContext(nc) as tc:
        with (
            tc.tile_pool(name="const", bufs=1) as const,
            tc.tile_pool(name="work", bufs=work_bufs) as work,
            tc.tile_pool(name="fin", bufs=2) as fin,
            tc.tile_pool(name="psum", bufs=psum_bufs, space="PSUM") as psum,
        ):
            # ---- weights / constants into SBUF (small K=2 operands first;
            # fp8 weights and hT[0..1] in fine-grained gate/bgrp chunks so
            # the first matmuls' prerequisites land early) ----
            nrep = NJ
            if y8:
                wb_sb = const.tile([(nrep - 1) * 32 + 1, 2, 4 * H], f8,
                                   name="wb_sb")
                yt_sb = const.tile([(nrep - 1) * 32 + 1, 2, T * b_loc], f8,
                                   name="yt_sb")
                for r in range(nrep):
                    nc.sync.dma_start(wb_sb[32 * r:32 * r + 1, :, :],
                                      wb8_in.ap())
                    nc.sync.dma_start(yt_sb[32 * r:32 * r + 1, :, :],
                                      yt_in.ap())
            else:
                wb_sb = const.tile([(nrep - 1) * 32 + 2, 4 * H], bf16,
                                   name="wb_sb")
                yt_sb = const.tile([(nrep - 1) * 32 + 2, T * b_loc], bf16,
                                   name="yt_sb")
                for r in range(nrep):
                    nc.sync.dma_start(wb_sb[32 * r:32 * r + 2, :], wb_in.ap())
                    nc.sync.dma_start(
                        yt_sb[32 * r:32 * r + 2, :],
                        yt_in.ap().rearrange("r t b -> r (t b)"))
            w_sb = []
            w8_ap = [w8_in.ap()[q].rearrange("p (i m) -> p i m", i=2)
                     for q in range(NQ)]
            for q in range(NQ):
                wt = const.tile([P, 2, 4 * H], f8, name=f"w8_{q}",
                                tag=f"w8_{q}")
                w_sb.append(wt)
            hT = []
            for t in range(T):
                ht = const.tile([P, NJ, b_loc], f8, name=f"hT{t}",
                                tag=f"hT{t}")
                hT.append(ht)
            # per-gate weight chunks in main-loop gate order (i, g, f, o),
            # interleaved with the first two timesteps' rhs halves
            for g in (G_I, G_G):
                for q in range(NQ):
                    gs = slice(g * H, (g + 1) * H)
                    nc.sync.dma_start(w_sb[q][:, :, gs], w8_ap[q][:, :, gs])
            nc.sync.dma_start(
                hT[0][:, :, 0:bgrp],
                hT_in.ap()[0].rearrange("p (k b) -> p k b", k=NJ)[:, :, 0:bgrp])
            for g in (G_F, G_O):
                for q in range(NQ):
                    gs = slice(g * H, (g + 1) * H)
                    nc.sync.dma_start(w_sb[q][:, :, gs], w8_ap[q][:, :, gs])
            nc.sync.dma_start(
                hT[1][:, :, 0:bgrp],
                hT_in.ap()[1].rearrange("p (k b) -> p k b", k=NJ)[:, :, 0:bgrp])
            # bulk loads go out on the second HWDGE ring (ACT sequencer) so
            # they don't queue behind the critical-path loads above
            for t in range(T):
                ap_t = hT_in.ap()[t].rearrange("p (k b) -> p k b", k=NJ)
                if t < 2:
                    nc.scalar.dma_start(hT[t][:, :, bgrp:b_loc],
                                        ap_t[:, :, bgrp:b_loc])
                else:
                    nc.scalar.dma_start(hT[t][:], ap_t)
            vd_sb = const.tile([P, NJ], bf16, name="vd_sb")
            nc.scalar.dma_start(vd_sb[:], vd_in.ap())
            vh_sb = const.tile([P, CH], bf16, name="vh_sb")
            nc.scalar.dma_start(vh_sb[:], vh_in.ap())
            h9_t = []
            for bt in range(NBT):
                h9 = const.tile([P, CH], bf16, name=f"h9_{bt}",
                                tag=f"h9_{bt}")
                nc.scalar.dma_start(h9[:], h9_in.ap()[bt * P:(bt + 1) * P, :])
                h9_t.append(h9)
            yh = const.tile([P, NBT], f32, name="yh")

            def k2_matmuls(ps, gate, t, bg):
                """K=2 bias/y matmuls: 4-up tile_position pack, start=True
                clears the 4 banks (each j-region is exactly one bank)."""
                for j in range(NJ):
                    mi = gate * NJ + j
                    if y8:
                        nc.tensor.matmul(
                            ps[:, j * bgrp:(j + 1) * bgrp],
                            wb_sb[32 * j:32 * j + 1, :,
                                  mi * P:(mi + 1) * P],
                            yt_sb[32 * j:32 * j + 1, :,
                                  t * b_loc + bg * bgrp:
                                  t * b_loc + (bg + 1) * bgrp],
                            start=True, stop=False,
                            tile_position=(32 * j, 0),
                            perf_mode=DR,
                            skip_group_check=True)
                    else:
                        nc.tensor.matmul(
                            ps[:, j * bgrp:(j + 1) * bgrp],
                            wb_sb[32 * j:32 * j + 2, mi * P:(mi + 1) * P],
                            yt_sb[32 * j:32 * j + 2,
                                  t * b_loc + bg * bgrp:
                                  t * b_loc + (bg + 1) * bgrp],
                            start=True, stop=False,
                            tile_position=(32 * j, 0),
                            skip_group_check=True)

            def dr_matmuls(ps, gate, t, bg):
                """fp8 DoubleRow matmuls, K=256 each (two 128-ch regions)."""
                for j in range(NJ):
                    mi = gate * NJ + j
                    for q in range(NQ):
                        nc.tensor.matmul(
                            ps[:, j * bgrp:(j + 1) * bgrp],
                            w_sb[q][:, :, mi * P:(mi + 1) * P],
                            hT[t][:, 2 * q:2 * q + 2,
                                  bg * bgrp:(bg + 1) * bgrp],
                            start=False, stop=(q == NQ - 1),
                            perf_mode=DR,
                            skip_group_check=True)

            def gate_matmul(gate, t, bg):
                ps = psum.tile([P, GW], f32, name="ps_big", tag="ps")
                k2_matmuls(ps, gate, t, bg)
                dr_matmuls(ps, gate, t, bg)
                return ps

            # ---- main loop ----
            # Per bg: 30 gate tiles in order [I0 G0 I1 G1 F1 ... I9 G9 F9 O9]
            # processed in PAIRS: both tiles' K=2 groups issue together, then
            # both DR batches — halving the PE tile-config switch cost.
            y_d_t = []
            c_bg = []
            so_bg = []
            for bg in range(NBG):
                c_t = const.tile([P, GW], bf16 if c_bf16 else f32,
                                 name=f"c_{bg}", tag=f"c{bg}")
                so_t = fin.tile([P, GW], bf16, name="so", tag=f"so{bg}",
                                bufs=1)
                so_bg.append(so_t)

                gate_list = [(G_I, 0), (G_G, 0)]
                for t in range(1, T):
                    gate_list += [(G_I, t), (G_G, t), (G_F, t)]
                gate_list.append((G_O, T - 1))

                tiles = {}      # (gate, t) -> work tile with ACT output

                def emit_consumers(gate, t, c_t=c_t, tiles=tiles):
                    """DVE ops that become ready once (gate, t) is activated.

                    Called right after the ACT emission for (gate, t); FIFO
                    order on DVE preserves the c-chain sequence."""
                    if gate == G_G:
                        si = tiles.pop((G_I, t))
                        tg = tiles.pop((G_G, t))
                        if t == 0:
                            nc.vector.tensor_tensor(c_t[:], si[:], tg[:],
                                                    ALU.mult)
                        else:
                            m = work.tile([P, GW], bf16, name="m", tag="m")
                            nc.vector.tensor_tensor(m[:], si[:], tg[:],
                                                    ALU.mult)
                            tiles[("m", t)] = m
                    elif gate == G_F:
                        sf = tiles.pop((G_F, t))
                        m = tiles.pop(("m", t))
                        nc.vector.tensor_tensor(c_t[:], c_t[:], sf[:],
                                                ALU.mult)
                        nc.vector.tensor_tensor(c_t[:], c_t[:], m[:],
                                                ALU.add)

                if pair_gates:
                    for k in range(0, len(gate_list), 2):
                        pair = gate_list[k:k + 2]
                        pss = []
                        for gate, t in pair:
                            ps = psum.tile([P, GW], f32, name="ps_big",
                                           tag="ps")
                            k2_matmuls(ps, gate, t, bg)
                            pss.append(ps)
                        for (gate, t), ps in zip(pair, pss):
                            dr_matmuls(ps, gate, t, bg)
                        for (gate, t), ps in zip(pair, pss):
                            if gate == G_O:
                                nc.scalar.activation(so_t[:], ps[:],
                                                     AF.Sigmoid, scale=INV)
                            else:
                                func = AF.Tanh if gate == G_G else AF.Sigmoid
                                wt = work.tile([P, GW], bf16, name="act",
                                               tag="si" if gate == G_I else
                                               ("tg" if gate == G_G else
                                                "sf"))
                                nc.scalar.activation(wt[:], ps[:], func,
                                                     scale=INV)
                                tiles[(gate, t)] = wt
                            emit_consumers(gate, t)
                else:
                    for gate, t in gate_list:
                        ps = gate_matmul(gate, t, bg)
                        if gate == G_O:
                            # per-j chunks: lets the finalize chain start
                            # ~1.3us earlier at the tail
                            for j in range(NJ):
                                sl = slice(j * bgrp, (j + 1) * bgrp)
                                nc.scalar.activation(so_t[:, sl], ps[:, sl],
                                                     AF.Sigmoid, scale=INV)
                        else:
                            func = AF.Tanh if gate == G_G else AF.Sigmoid
                            wt = work.tile([P, GW], bf16, name="act",
                                           tag="si" if gate == G_I else
                                           ("tg" if gate == G_G else "sf"))
                            nc.scalar.activation(wt[:], ps[:], func,
                                                 scale=INV)
                            tiles[(gate, t)] = wt
                        emit_consumers(gate, t)

                c_bg.append(c_t)
                # y_h = h9 . v_h for this group's rows on DVE (has slack
                # while the other group computes)
                for u in range(NBT // NBG):
                    bt = bg * (NBT // NBG) + u
                    tmp = work.tile([P, CH], bf16, name="tmp9", tag="tmp9")
                    if use_ttr:
                        nc.vector.tensor_tensor_reduce(
                            tmp[:], h9_t[bt][:], vh_sb[:], 1.0, 0.0,
                            ALU.mult, ALU.add, yh[:, bt:bt + 1])
                    else:
                        nc.vector.tensor_tensor(tmp[:], h9_t[bt][:],
                                                vh_sb[:], ALU.mult)
                        nc.vector.tensor_reduce(yh[:, bt:bt + 1], tmp[:],
                                                mybir.AxisListType.X,
                                                ALU.add)

                # d = sigma(o) * tanh(c) per j-chunk now (ACT/DVE only, no
                # PSUM — bg0's part overlaps bg1's main loop); dots deferred
                d_bg = []
                for j in range(NJ):
                    sl = slice(j * bgrp, (j + 1) * bgrp)
                    tc9 = fin.tile([P, bgrp], bf16, name="tc9", tag="tc9",
                                   bufs=3)
                    nc.scalar.activation(tc9[:], c_t[:, sl], AF.Tanh)
                    d = fin.tile([P, bgrp], bf16, name="d", tag=f"d{bg}_{j}",
                                 bufs=1)
                    nc.vector.tensor_tensor(d[:], so_t[:, sl], tc9[:],
                                            ALU.mult)
                    d_bg.append(d)
                y_d_t.append(d_bg)

            # ---- final dots (after all gate psum tiles -> no rotation
            # stalls) ----
            for bg in range(NBG):
                ps_y = psum.tile([1, bgrp], f32, name="ps_y", tag="ps")
                for j in range(NJ):
                    nc.tensor.matmul(ps_y[:], vd_sb[:, j:j + 1],
                                     y_d_t[bg][j][:],
                                     start=(j == 0), stop=(j == NJ - 1))
                y_d = fin.tile([1, bgrp], f32, name="y_d", tag=f"y_d{bg}",
                               bufs=1)
                nc.vector.tensor_copy(y_d[:], ps_y[:])
                nc.sync.dma_start(outd.ap()[bg * bgrp:(bg + 1) * bgrp],
                                  y_d[:])

            nc.sync.dma_start(outh.ap(), yh[:])

    nc.compile()
    return nc


def _host_prep(inputs):
    W_hh = np.asarray(inputs["W_hh"], np.float32)
    W_ih = np.asarray(inputs["W_ih"], np.float32)
    b = (np.asarray(inputs["b_ih"], np.float32)
         + np.asarray(inputs["b_hh"], np.float32))          # [2048]
    fc1_w = np.asarray(inputs["fc1_w"], np.float32)
    fc2_w = np.asarray(inputs["fc2_w"], np.float32)
    v = (fc2_w @ fc1_w)[0]                                   # [1024]
    c0 = float(np.asarray(inputs["fc1_b"], np.float32) @ fc2_w[0]
               + np.asarray(inputs["fc2_b"], np.float32)[0])

    NJ = H // P
    # w8[q, p, i, m] = fp8(W_hh[m, 128*(2q+i) + p] * S)
    W8T = (W_hh * S).astype(F8).T                            # [512, 2048]
    w8 = np.ascontiguousarray(
        W8T.reshape(2, 2, P, 4 * H).transpose(0, 2, 1, 3)
    ).reshape(2, P, 2 * 4 * H)
    wb_f32 = np.stack([W_ih[:, 0] * S, b * S])               # [2, 2048]
    if Y8:
        wih_b = np.ascontiguousarray(wb_f32.astype(F8))[None]
    else:
        wih_b = np.ascontiguousarray(wb_f32.astype(BF16))
    v_d = np.ascontiguousarray(v[:H].reshape(NJ, P).T.copy())       # [128,4]
    v_h = np.ascontiguousarray(np.tile(v[H:][None, :], (P, 1)))     # [128,512]
    return w8, wih_b, v_d.astype(BF16), v_h.astype(BF16), c0


def _install_ntff_shim():
    """Best-effort: recreate antenv.axon_hooks so trace=True can profile."""
    import sys as _sys
    import types as _types
    try:
        import antenv.axon_hooks  # noqa: F401
        return
    except ImportError:
        pass
    try:
        import antenv
        from trn_agent_boot.trn_boot import _ntff_profile_via_ctypes
        hook = _ntff_profile_via_ctypes("/opt/axon/libaxon_pjrt.so")
        mod = _types.ModuleType("antenv.axon_hooks")
        _state = {"hook": hook}
        mod.set_axon_ntff_profile_hook = lambda hk: _state.__setitem__("hook", hk)
        mod.get_axon_ntff_profile_hook = lambda: _state["hook"]
        _sys.modules["antenv.axon_hooks"] = mod
        antenv.axon_hooks = mod
    except Exception:
        pass


def make_in_maps(inputs):
    w8, wih_b, v_d, v_h, c0 = _host_prep(inputs)
    h = np.asarray(inputs["h"], np.float32)
    y = np.asarray(inputs["y_seq"], np.float32)
    NJ = H // P
    in_maps = []
    for c in range(N_CORES):
        sl = slice(c * B_LOC, (c + 1) * B_LOC)
        h_sl = h[sl]                                         # [1024, 10, 512]
        h8 = h_sl.astype(F8)
        # hT8[t, p, k*1024 + b] = fp8(h[b, t, 128k + p])
        hT8 = np.ascontiguousarray(
            h8.transpose(1, 2, 0)                            # [T, CH, b_loc]
            .reshape(T, NJ, P, B_LOC)
            .transpose(0, 2, 1, 3)                           # [T, P, NJ, b]
        ).reshape(T, P, NJ * B_LOC)
        if Y8:
            yt = np.empty((1, 2, T * B_LOC), F8)
            yt[0, 0] = y[sl].T.astype(F8).reshape(-1)
            yt[0, 1] = np.ones(T * B_LOC, F8)
            wb_key = "wb8"
        else:
            yt = np.empty((2, T, B_LOC), BF16)
            yt[0] = y[sl].T.astype(BF16)
            yt[1] = np.ones((T, B_LOC), BF16)
            wb_key = "wih_b"
        in_maps.append({
            "hT8": hT8,
            "h9": np.ascontiguousarray(h_sl[:, T - 1, :].astype(BF16)),
            "yt_aug": yt,
            "w8": w8, wb_key: wih_b,
            "v_d": v_d, "v_h": v_h,
        })
    return in_maps, c0


def run(inputs, trace=False):
    key = "full"
    if key not in _compiled:
        _compiled[key] = build_nc()
    nc = _compiled[key]

    if trace:
        _install_ntff_shim()

    in_maps, c0 = make_in_maps(inputs)
    res = run_bass_kernel_spmd(nc, in_maps, core_ids=list(range(N_CORES)),
                               trace=trace)
    outs = []
    for c in range(N_CORES):
        r = res.results[c]
        y_core = (r["out_d"] + r["out_h"].T.reshape(-1) + c0)
        outs.append(y_core.astype(np.float32))
    return np.concatenate(outs)[:, None], res


def kernel(**inputs):
    out, _ = run(inputs, trace=False)
    return out


# revision 32
# speedup vs baseline: 1.0964x; 1.0964x over previous
"""Trainium2 Bass kernel for nn_AttnDecoder (B=8192, T=10, CH=H=512).

Math notes (verified against the jax reference in fp32 to ~3e-6):
  - The attention block is dead code: softmax over a size-1 axis == 1, so
    h1 == ht and attn1/2/3 never affect the output.
  - The LSTM hidden state d never feeds back into the gates (only the cell
    state c does, elementwise), so the only sequential part is
        c_t = sigmoid(f_t) * c_{t-1} + sigmoid(i_t) * tanh(g_t)
    a cheap elementwise recurrence over T=10.
  - o-gate is only needed at t = T-1.
  - fc2(fc1(z)) with no nonlinearity folds into a single vector:
        y = d . v[:H] + h9 . v[H:] + c0,   v = (fc2_w @ fc1_w)^T.

Sharding: batch-parallel over 8 cores (1024 rows each), weights replicated.

Implementation (v2, fp8 DoubleRow):
  - h is pre-cast to fp8-e4m3 and pre-transposed on the host into
    hT8[t, p, k*1024+b] = fp8(h[b, t, 128k+p]); whole-tensor rel-err budget
    allows it (fp8 pipeline sims at 8.3e-3 vs the 2e-2 gate).
  - Gate matmuls run in fp8 DoubleRow mode: each instruction contracts
    K=256 (two 128-channel regions, lhsT/rhs APs shaped [128, 2, n]),
    halving PE streaming time vs bf16.
  - Weights are pre-scaled by S=16 on the host (better fp8 mantissa use);
    the activation instruction compensates with scale=1/S for free.
  - y_t / bias enter via a K=2 bf16 matmul (rows [y;1] x [w_ih*S; b*S]),
    4-up tile_position-packed, accumulating into the same PSUM banks.
  - Cell state c is bf16 (DVE 2x mode); sim shows no accuracy change.
  - Everything is resident in SBUF up-front (~120KB/partition): no SWDGE,
    no DRAM staging, no transpose DMAs, no gpsimd.
  - Final: d = sigma(o)*tanh(c); y_d via fp32 PE dot with v[:H];
    y_h = h9 . v[H:] fused mult+reduce on DVE; partials summed on host.
"""

import numpy as np
import ml_dtypes

import concourse.bass as bass
import concourse.tile as tile
from concourse import bacc, mybir
from concourse.bass_utils import run_bass_kernel_spmd

BF16 = ml_dtypes.bfloat16
F8 = ml_dtypes.float8_e4m3

B, T, CH, H = 8192, 10, 512, 512
N_CORES = 8
B_LOC = B // N_CORES            # 1024 batch rows per core
P = 128
S = 16.0                        # fp8 weight pre-scale
Y8 = False                      # y/bias matmul operands in fp8 DR form

_compiled = {}


def build_nc(b_loc=B_LOC, bgrp=512, psum_bufs=2, work_bufs=2,
             use_ttr=False, c_bf16=True, use_dr=True, y8=Y8,
             pair_gates=False):
    NBG = b_loc // bgrp         # batch groups
    NJ = H // P                 # 4 hid chunks
    NQ = CH // (2 * P)          # 2 DoubleRow K-chunks (256 channels each)
    NBT = b_loc // P            # batch tiles for the h9 dot
    GW = NJ * bgrp              # big-tile width (one gate, all hid chunks)
    f32 = mybir.dt.float32
    bf16 = mybir.dt.bfloat16
    f8 = mybir.dt.float8e4
    AF = mybir.ActivationFunctionType
    ALU = mybir.AluOpType
    DR = mybir.MatmulPerfMode.DoubleRow

    nc = bacc.Bacc("TRN2", target_bir_lowering=False, debug=False,
                   num_devices=N_CORES)

    # hT8[t, p, k*b_loc + b] = fp8(h[b, t, 128k + p])
    hT_in = nc.dram_tensor("hT8", [T, P, NJ * b_loc], f8, kind="ExternalInput")
    h9_in = nc.dram_tensor("h9", [b_loc, CH], bf16, kind="ExternalInput")
    # per t: rhs rows [y_t ; ones] for the K=2 bias/y matmul
    if y8:
        yt_in = nc.dram_tensor("yt_aug", [1, 2, T * b_loc], f8,
                               kind="ExternalInput")
        wb8_in = nc.dram_tensor("wb8", [1, 2, 4 * H], f8,
                                kind="ExternalInput")
    else:
        yt_in = nc.dram_tensor("yt_aug", [2, T, b_loc], bf16,
                               kind="ExternalInput")
    # w8[q, p, i*2048 + m] = fp8(W_hh[m, 128*(2q+i) + p] * S)
    w8_in = nc.dram_tensor("w8", [NQ, P, 2 * 4 * H], f8, kind="ExternalInput")
    if not y8:
        # rows: [w_ih * S ; (b_ih + b_hh) * S]
        wb_in = nc.dram_tensor("wih_b", [2, 4 * H], bf16,
                               kind="ExternalInput")
    vd_in = nc.dram_tensor("v_d", [P, NJ], bf16, kind="ExternalInput")
    vh_in = nc.dram_tensor("v_h", [P, CH], bf16, kind="ExternalInput")
    outd = nc.dram_tensor("out_d", [b_loc], f32, kind="ExternalOutput")
    outh = nc.dram_tensor("out_h", [P, NBT], f32, kind="ExternalOutput")

    G_I, G_F, G_G, G_O = 0, 1, 2, 3     # gate blocks in the 2048 W columns
    INV = 1.0 / S

    with tile.TileContext(nc) as tc:
        with (
            tc.tile_pool(name="const", bufs=1) as const,
            tc.tile_pool(name="work", bufs=work_bufs) as work,
            tc.tile_pool(name="fin", bufs=2) as fin,
            tc.tile_pool(name="psum", bufs=psum_bufs, space="PSUM") as psum,
        ):
            # ---- weights / constants into SBUF (small K=2 operands first;
            # fp8 weights and hT[0..1] in fine-grained gate/bgrp chunks so
            # the first matmuls' prerequisites land early) ----
            nrep = NJ
            if y8:
                wb_sb = const.tile([(nrep - 1) * 32 + 1, 2, 4 * H], f8,
                                   name="wb_sb")
                yt_sb = const.tile([(nrep - 1) * 32 + 1, 2, T * b_loc], f8,
                                   name="yt_sb")
                for r in range(nrep):
                    nc.sync.dma_start(wb_sb[32 * r:32 * r + 1, :, :],
                                      wb8_in.ap())
                    nc.sync.dma_start(yt_sb[32 * r:32 * r + 1, :, :],
                                      yt_in.ap())
            else:
                wb_sb = const.tile([(nrep - 1) * 32 + 2, 4 * H], bf16,
                                   name="wb_sb")
                yt_sb = const.tile([(nrep - 1) * 32 + 2, T * b_loc], bf16,
                                   name="yt_sb")
                for r in range(nrep):
                    nc.sync.dma_start(wb_sb[32 * r:32 * r + 2, :], wb_in.ap())
                    nc.sync.dma_start(
                        yt_sb[32 * r:32 * r + 2, :],
                        yt_in.ap().rearrange("r t b -> r (t b)"))
            w_sb = []
            w8_ap = [w8_in.ap()[q].rearrange("p (i m) -> p i m", i=2)
                     for q in range(NQ)]
            for q in range(NQ):
                wt = const.tile([P, 2, 4 * H], f8, name=f"w8_{q}",
                                tag=f"w8_{q}")
                w_sb.append(wt)
            hT = []
            for t in range(T):
                ht = const.tile([P, NJ, b_loc], f8, name=f"hT{t}",
                                tag=f"hT{t}")
                hT.append(ht)
            # per-gate weight chunks in main-loop gate order (i, g, f, o),
            # interleaved with the first two timesteps' rhs halves
            for g in (G_I, G_G):
                for q in range(NQ):
                    gs = slice(g * H, (g + 1) * H)
                    nc.sync.dma_start(w_sb[q][:, :, gs], w8_ap[q][:, :, gs])
            nc.sync.dma_start(
                hT[0][:, :, 0:bgrp],
                hT_in.ap()[0].rearrange("p (k b) -> p k b", k=NJ)[:, :, 0:bgrp])
            for g in (G_F, G_O):
                for q in range(NQ):
                    gs = slice(g * H, (g + 1) * H)
                    nc.sync.dma_start(w_sb[q][:, :, gs], w8_ap[q][:, :, gs])
            nc.sync.dma_start(
                hT[1][:, :, 0:bgrp],
                hT_in.ap()[1].rearrange("p (k b) -> p k b", k=NJ)[:, :, 0:bgrp])
            for t in range(T):
                ap_t = hT_in.ap()[t].rearrange("p (k b) -> p k b", k=NJ)
                if t < 2:
                    nc.sync.dma_start(hT[t][:, :, bgrp:b_loc],
                                      ap_t[:, :, bgrp:b_loc])
                else:
                    nc.sync.dma_start(hT[t][:], ap_t)
            vd_sb = const.tile([P, NJ], bf16, name="vd_sb")
            nc.sync.dma_start(vd_sb[:], vd_in.ap())
            vh_sb = const.tile([P, CH], bf16, name="vh_sb")
            nc.sync.dma_start(vh_sb[:], vh_in.ap())
            h9_t = []
            for bt in range(NBT):
                h9 = const.tile([P, CH], bf16, name=f"h9_{bt}",
                                tag=f"h9_{bt}")
                nc.sync.dma_start(h9[:], h9_in.ap()[bt * P:(bt + 1) * P, :])
                h9_t.append(h9)
            yh = const.tile([P, NBT], f32, name="yh")

            def k2_matmuls(ps, gate, t, bg):
                """K=2 bias/y matmuls: 4-up tile_position pack, start=True
                clears the 4 banks (each j-region is exactly one bank)."""
                for j in range(NJ):
                    mi = gate * NJ + j
                    if y8:
                        nc.tensor.matmul(
                            ps[:, j * bgrp:(j + 1) * bgrp],
                            wb_sb[32 * j:32 * j + 1, :,
                                  mi * P:(mi + 1) * P],
                            yt_sb[32 * j:32 * j + 1, :,
                                  t * b_loc + bg * bgrp:
                                  t * b_loc + (bg + 1) * bgrp],
                            start=True, stop=False,
                            tile_position=(32 * j, 0),
                            perf_mode=DR,
                            skip_group_check=True)
                    else:
                        nc.tensor.matmul(
                            ps[:, j * bgrp:(j + 1) * bgrp],
                            wb_sb[32 * j:32 * j + 2, mi * P:(mi + 1) * P],
                            yt_sb[32 * j:32 * j + 2,
                                  t * b_loc + bg * bgrp:
                                  t * b_loc + (bg + 1) * bgrp],
                            start=True, stop=False,
                            tile_position=(32 * j, 0),
                            skip_group_check=True)

            def dr_matmuls(ps, gate, t, bg):
                """fp8 DoubleRow matmuls, K=256 each (two 128-ch regions)."""
                for j in range(NJ):
                    mi = gate * NJ + j
                    for q in range(NQ):
                        nc.tensor.matmul(
                            ps[:, j * bgrp:(j + 1) * bgrp],
                            w_sb[q][:, :, mi * P:(mi + 1) * P],
                            hT[t][:, 2 * q:2 * q + 2,
                                  bg * bgrp:(bg + 1) * bgrp],
                            start=False, stop=(q == NQ - 1),
                            perf_mode=DR,
                            skip_group_check=True)

            def gate_matmul(gate, t, bg):
                ps = psum.tile([P, GW], f32, name="ps_big", tag="ps")
                k2_matmuls(ps, gate, t, bg)
                dr_matmuls(ps, gate, t, bg)
                return ps

            # ---- main loop ----
            # Per bg: 30 gate tiles in order [I0 G0 I1 G1 F1 ... I9 G9 F9 O9]
            # processed in PAIRS: both tiles' K=2 groups issue together, then
            # both DR batches — halving the PE tile-config switch cost.
            y_d_t = []
            c_bg = []
            so_bg = []
            for bg in range(NBG):
                c_t = const.tile([P, GW], bf16 if c_bf16 else f32,
                                 name=f"c_{bg}", tag=f"c{bg}")
                so_t = fin.tile([P, GW], bf16, name="so", tag=f"so{bg}",
                                bufs=1)
                so_bg.append(so_t)

                gate_list = [(G_I, 0), (G_G, 0)]
                for t in range(1, T):
                    gate_list += [(G_I, t), (G_G, t), (G_F, t)]
                gate_list.append((G_O, T - 1))

                tiles = {}      # (gate, t) -> work tile with ACT output

                def emit_consumers(gate, t, c_t=c_t, tiles=tiles):
                    """DVE ops that become ready once (gate, t) is activated.

                    Called right after the ACT emission for (gate, t); FIFO
                    order on DVE preserves the c-chain sequence."""
                    if gate == G_G:
                        si = tiles.pop((G_I, t))
                        tg = tiles.pop((G_G, t))
                        if t == 0:
                            nc.vector.tensor_tensor(c_t[:], si[:], tg[:],
                                                    ALU.mult)
                        else:
                            m = work.tile([P, GW], bf16, name="m", tag="m")
                            nc.vector.tensor_tensor(m[:], si[:], tg[:],
                                                    ALU.mult)
                            tiles[("m", t)] = m
                    elif gate == G_F:
                        sf = tiles.pop((G_F, t))
                        m = tiles.pop(("m", t))
                        nc.vector.tensor_tensor(c_t[:], c_t[:], sf[:],
                                                ALU.mult)
                        nc.vector.tensor_tensor(c_t[:], c_t[:], m[:],
                                                ALU.add)

                if pair_gates:
                    for k in range(0, len(gate_list), 2):
                        pair = gate_list[k:k + 2]
                        pss = []
                        for gate, t in pair:
                            ps = psum.tile([P, GW], f32, name="ps_big",
                                           tag="ps")
                            k2_matmuls(ps, gate, t, bg)
                            pss.append(ps)
                        for (gate, t), ps in zip(pair, pss):
                            dr_matmuls(ps, gate, t, bg)
                        for (gate, t), ps in zip(pair, pss):
                            if gate == G_O:
                                nc.scalar.activation(so_t[:], ps[:],
                                                     AF.Sigmoid, scale=INV)
                            else:
                                func = AF.Tanh if gate == G_G else AF.Sigmoid
                                wt = work.tile([P, GW], bf16, name="act",
                                               tag="si" if gate == G_I else
                                               ("tg" if gate == G_G else
                                                "sf"))
                                nc.scalar.activation(wt[:], ps[:], func,
                                                     scale=INV)
                                tiles[(gate, t)] = wt
                            emit_consumers(gate, t)
                else:
                    for gate, t in gate_list:
                        ps = gate_matmul(gate, t, bg)
                        if gate == G_O:
                            # per-j chunks: lets the finalize chain start
                            # ~1.3us earlier at the tail
                            for j in range(NJ):
                                sl = slice(j * bgrp, (j + 1) * bgrp)
                                nc.scalar.activation(so_t[:, sl], ps[:, sl],
                                                     AF.Sigmoid, scale=INV)
                        else:
                            func = AF.Tanh if gate == G_G else AF.Sigmoid
                            wt = work.tile([P, GW], bf16, name="act",
                                           tag="si" if gate == G_I else
                                           ("tg" if gate == G_G else "sf"))
                            nc.scalar.activation(wt[:], ps[:], func,
                                                 scale=INV)
                            tiles[(gate, t)] = wt
                        emit_consumers(gate, t)

                c_bg.append(c_t)
                # y_h = h9 . v_h for this group's rows on DVE (has slack
                # while the other group computes)
                for u in range(NBT // NBG):
                    bt = bg * (NBT // NBG) + u
                    tmp = work.tile([P, CH], bf16, name="tmp9", tag="tmp9")
                    if use_ttr:
                        nc.vector.tensor_tensor_reduce(
                            tmp[:], h9_t[bt][:], vh_sb[:], 1.0, 0.0,
                            ALU.mult, ALU.add, yh[:, bt:bt + 1])
                    else:
                        nc.vector.tensor_tensor(tmp[:], h9_t[bt][:],
                                                vh_sb[:], ALU.mult)
                        nc.vector.tensor_reduce(yh[:, bt:bt + 1], tmp[:],
                                                mybir.AxisListType.X,
                                                ALU.add)

                # d = sigma(o) * tanh(c) per j-chunk now (ACT/DVE only, no
                # PSUM — bg0's part overlaps bg1's main loop); dots deferred
                d_bg = []
                for j in range(NJ):
                    sl = slice(j * bgrp, (j + 1) * bgrp)
                    tc9 = fin.tile([P, bgrp], bf16, name="tc9", tag="tc9",
                                   bufs=3)
                    nc.scalar.activation(tc9[:], c_t[:, sl], AF.Tanh)
                    d = fin.tile([P, bgrp], bf16, name="d", tag=f"d{bg}_{j}",
                                 bufs=1)
                    nc.vector.tensor_tensor(d[:], so_t[:, sl], tc9[:],
                                            ALU.mult)
                    d_bg.append(d)
                y_d_t.append(d_bg)

            # ---- final dots (after all gate psum tiles -> no rotation
            # stalls) ----
            for bg in range(NBG):
                ps_y = psum.tile([1, bgrp], f32, name="ps_y", tag="ps")
                for j in range(NJ):
                    nc.tensor.matmul(ps_y[:], vd_sb[:, j:j + 1],
                                     y_d_t[bg][j][:],
                                     start=(j == 0), stop=(j == NJ - 1))
                y_d = fin.tile([1, bgrp], f32, name="y_d", tag=f"y_d{bg}",
                               bufs=1)
                nc.vector.tensor_copy(y_d[:], ps_y[:])
                nc.sync.dma_start(outd.ap()[bg * bgrp:(bg + 1) * bgrp],
                                  y_d[:])

            nc.sync.dma_start(outh.ap(), yh[:])

    nc.compile()
    return nc


def _host_prep(inputs):
    W_hh = np.asarray(inputs["W_hh"], np.float32)
    W_ih = np.asarray(inputs["W_ih"], np.float32)
    b = (np.asarray(inputs["b_ih"], np.float32)
         + np.asarray(inputs["b_hh"], np.float32))          # [2048]
    fc1_w = np.asarray(inputs["fc1_w"], np.float32)
    fc2_w = np.asarray(inputs["fc2_w"], np.float32)
    v = (fc2_w @ fc1_w)[0]                                   # [1024]
    c0 = float(np.asarray(inputs["fc1_b"], np.float32) @ fc2_w[0]
               + np.asarray(inputs["fc2_b"], np.float32)[0])

    NJ = H // P
    # w8[q, p, i, m] = fp8(W_hh[m, 128*(2q+i) + p] * S)
    W8T = (W_hh * S).astype(F8).T                            # [512, 2048]
    w8 = np.ascontiguousarray(
        W8T.reshape(2, 2, P, 4 * H).transpose(0, 2, 1, 3)
    ).reshape(2, P, 2 * 4 * H)
    wb_f32 = np.stack([W_ih[:, 0] * S, b * S])               # [2, 2048]
    if Y8:
        wih_b = np.ascontiguousarray(wb_f32.astype(F8))[None]
    else:
        wih_b = np.ascontiguousarray(wb_f32.astype(BF16))
    v_d = np.ascontiguousarray(v[:H].reshape(NJ, P).T.copy())       # [128,4]
    v_h = np.ascontiguousarray(np.tile(v[H:][None, :], (P, 1)))     # [128,512]
    return w8, wih_b, v_d.astype(BF16), v_h.astype(BF16), c0


def _install_ntff_shim():
    """Best-effort: recreate antenv.axon_hooks so trace=True can profile."""
    import sys as _sys
    import types as _types
    try:
        import antenv.axon_hooks  # noqa: F401
        return
    except ImportError:
        pass
    try:
        import antenv
        from trn_agent_boot.trn_boot import _ntff_profile_via_ctypes
        hook = _ntff_profile_via_ctypes("/opt/axon/libaxon_pjrt.so")
        mod = _types.ModuleType("antenv.axon_hooks")
        _state = {"hook": hook}
        mod.set_axon_ntff_profile_hook = lambda hk: _state.__setitem__("hook", hk)
        mod.get_axon_ntff_profile_hook = lambda: _state["hook"]
        _sys.modules["antenv.axon_hooks"] = mod
        antenv.axon_hooks = mod
    except Exception:
        pass


def make_in_maps(inputs):
    w8, wih_b, v_d, v_h, c0 = _host_prep(inputs)
    h = np.asarray(inputs["h"], np.float32)
    y = np.asarray(inputs["y_seq"], np.float32)
    NJ = H // P
    in_maps = []
    for c in range(N_CORES):
        sl = slice(c * B_LOC, (c + 1) * B_LOC)
        h_sl = h[sl]                                         # [1024, 10, 512]
        h8 = h_sl.astype(F8)
        # hT8[t, p, k*1024 + b] = fp8(h[b, t, 128k + p])
        hT8 = np.ascontiguousarray(
            h8.transpose(1, 2, 0)                            # [T, CH, b_loc]
            .reshape(T, NJ, P, B_LOC)
            .transpose(0, 2, 1, 3)                           # [T, P, NJ, b]
        ).reshape(T, P, NJ * B_LOC)
        if Y8:
            yt = np.empty((1, 2, T * B_LOC), F8)
            yt[0, 0] = y[sl].T.astype(F8).reshape(-1)
            yt[0, 1] = np.ones(T * B_LOC, F8)
            wb_key = "wb8"
        else:
            yt = np.empty((2, T, B_LOC), BF16)
            yt[0] = y[sl].T.astype(BF16)
            yt[1] = np.ones((T, B_LOC), BF16)
            wb_key = "wih_b"
        in_maps.append({
            "hT8": hT8,
            "h9": np.ascontiguousarray(h_sl[:, T - 1, :].astype(BF16)),
            "yt_aug": yt,
            "w8": w8, wb_key: wih_b,
            "v_d": v_d, "v_h": v_h,
        })
    return in_maps, c0


def run(inputs, trace=False):
    key = "full"
    if key not in _compiled:
        _compiled[key] = build_nc()
    nc = _compiled[key]

    if trace:
        _install_ntff_shim()

    in_maps, c0 = make_in_maps(inputs)
    res = run_bass_kernel_spmd(nc, in_maps, core_ids=list(range(N_CORES)),
                               trace=trace)
    outs = []
    for c in range(N_CORES):
        r = res.results[c]
        y_core = (r["out_d"] + r["out_h"].T.reshape(-1) + c0)
        outs.append(y_core.astype(np.float32))
    return np.concatenate(outs)[:, None], res


def kernel(**inputs):
    out, _ = run(inputs, trace=False)
    return out


# revision 33
# speedup vs baseline: 1.1002x; 1.0035x over previous
"""Trainium2 Bass kernel for nn_AttnDecoder (B=8192, T=10, CH=H=512).

Math notes (verified against the jax reference in fp32 to ~3e-6):
  - The attention block is dead code: softmax over a size-1 axis == 1, so
    h1 == ht and attn1/2/3 never affect the output.
  - The LSTM hidden state d never feeds back into the gates (only the cell
    state c does, elementwise), so the only sequential part is
        c_t = sigmoid(f_t) * c_{t-1} + sigmoid(i_t) * tanh(g_t)
    a cheap elementwise recurrence over T=10.
  - o-gate is only needed at t = T-1.
  - fc2(fc1(z)) with no nonlinearity folds into a single vector:
        y = d . v[:H] + h9 . v[H:] + c0,   v = (fc2_w @ fc1_w)^T.

Sharding: batch-parallel over 8 cores (1024 rows each), weights replicated.

Implementation (v2, fp8 DoubleRow):
  - h is pre-cast to fp8-e4m3 and pre-transposed on the host into
    hT8[t, p, k*1024+b] = fp8(h[b, t, 128k+p]); whole-tensor rel-err budget
    allows it (fp8 pipeline sims at 8.3e-3 vs the 2e-2 gate).
  - Gate matmuls run in fp8 DoubleRow mode: each instruction contracts
    K=256 (two 128-channel regions, lhsT/rhs APs shaped [128, 2, n]),
    halving PE streaming time vs bf16.
  - Weights are pre-scaled by S=16 on the host (better fp8 mantissa use);
    the activation instruction compensates with scale=1/S for free.
  - y_t / bias enter via a K=2 bf16 matmul (rows [y;1] x [w_ih*S; b*S]),
    4-up tile_position-packed, accumulating into the same PSUM banks.
  - Cell state c is bf16 (DVE 2x mode); sim shows no accuracy change.
  - Everything is resident in SBUF up-front (~120KB/partition): no SWDGE,
    no DRAM staging, no transpose DMAs, no gpsimd.
  - Final: d = sigma(o)*tanh(c); y_d via fp32 PE dot with v[:H];
    y_h = h9 . v[H:] fused mult+reduce on DVE; partials summed on host.
"""

import numpy as np
import ml_dtypes

import concourse.bass as bass
import concourse.tile as tile
from concourse import bacc, mybir
from concourse.bass_utils import run_bass_kernel_spmd

BF16 = ml_dtypes.bfloat16
F8 = ml_dtypes.float8_e4m3

B, T, CH, H = 8192, 10, 512, 512
N_CORES = 8
B_LOC = B // N_CORES            # 1024 batch rows per core
P = 128
S = 16.0                        # fp8 weight pre-scale
Y8 = False                      # y/bias matmul operands in fp8 DR form

_compiled = {}


def build_nc(b_loc=B_LOC, bgrp=512, psum_bufs=2, work_bufs=3,
             use_ttr=False, c_bf16=True, use_dr=True, y8=Y8,
             pair_gates=False):
    NBG = b_loc // bgrp         # batch groups
    NJ = H // P                 # 4 hid chunks
    NQ = CH // (2 * P)          # 2 DoubleRow K-chunks (256 channels each)
    NBT = b_loc // P            # batch tiles for the h9 dot
    GW = NJ * bgrp              # big-tile width (one gate, all hid chunks)
    f32 = mybir.dt.float32
    bf16 = mybir.dt.bfloat16
    f8 = mybir.dt.float8e4
    AF = mybir.ActivationFunctionType
    ALU = mybir.AluOpType
    DR = mybir.MatmulPerfMode.DoubleRow

    nc = bacc.Bacc("TRN2", target_bir_lowering=False, debug=False,
                   num_devices=N_CORES)

    # hT8[t, p, k*b_loc + b] = fp8(h[b, t, 128k + p])
    hT_in = nc.dram_tensor("hT8", [T, P, NJ * b_loc], f8, kind="ExternalInput")
    h9_in = nc.dram_tensor("h9", [b_loc, CH], bf16, kind="ExternalInput")
    # per t: rhs rows [y_t ; ones] for the K=2 bias/y matmul
    if y8:
        yt_in = nc.dram_tensor("yt_aug", [1, 2, T * b_loc], f8,
                               kind="ExternalInput")
        wb8_in = nc.dram_tensor("wb8", [1, 2, 4 * H], f8,
                                kind="ExternalInput")
    else:
        yt_in = nc.dram_tensor("yt_aug", [2, T, b_loc], bf16,
                               kind="ExternalInput")
    # w8[q, p, i*2048 + m] = fp8(W_hh[m, 128*(2q+i) + p] * S)
    w8_in = nc.dram_tensor("w8", [NQ, P, 2 * 4 * H], f8, kind="ExternalInput")
    if not y8:
        # rows: [w_ih * S ; (b_ih + b_hh) * S]
        wb_in = nc.dram_tensor("wih_b", [2, 4 * H], bf16,
                               kind="ExternalInput")
    vd_in = nc.dram_tensor("v_d", [P, NJ], bf16, kind="ExternalInput")
    vh_in = nc.dram_tensor("v_h", [P, CH], bf16, kind="ExternalInput")
    outd = nc.dram_tensor("out_d", [b_loc], f32, kind="ExternalOutput")
    outh = nc.dram_tensor("out_h", [P, NBT], f32, kind="ExternalOutput")

    G_I, G_F, G_G, G_O = 0, 1, 2, 3     # gate blocks in the 2048 W columns
    INV = 1.0 / S

    with tile.TileContext(nc) as tc:
        with (
            tc.tile_pool(name="const", bufs=1) as const,
            tc.tile_pool(name="work", bufs=work_bufs) as work,
            tc.tile_pool(name="fin", bufs=2) as fin,
            tc.tile_pool(name="psum", bufs=psum_bufs, space="PSUM") as psum,
        ):
            # ---- weights / constants into SBUF (small K=2 operands first;
            # fp8 weights and hT[0..1] in fine-grained gate/bgrp chunks so
            # the first matmuls' prerequisites land early) ----
            nrep = NJ
            if y8:
                wb_sb = const.tile([(nrep - 1) * 32 + 1, 2, 4 * H], f8,
                                   name="wb_sb")
                yt_sb = const.tile([(nrep - 1) * 32 + 1, 2, T * b_loc], f8,
                                   name="yt_sb")
                for r in range(nrep):
                    nc.sync.dma_start(wb_sb[32 * r:32 * r + 1, :, :],
                                      wb8_in.ap())
                    nc.sync.dma_start(yt_sb[32 * r:32 * r + 1, :, :],
                                      yt_in.ap())
            else:
                wb_sb = const.tile([(nrep - 1) * 32 + 2, 4 * H], bf16,
                                   name="wb_sb")
                yt_sb = const.tile([(nrep - 1) * 32 + 2, T * b_loc], bf16,
                                   name="yt_sb")
                for r in range(nrep):
                    nc.sync.dma_start(wb_sb[32 * r:32 * r + 2, :], wb_in.ap())
                    nc.sync.dma_start(
                        yt_sb[32 * r:32 * r + 2, :],
                        yt_in.ap().rearrange("r t b -> r (t b)"))
            w_sb = []
            w8_ap = [w8_in.ap()[q].rearrange("p (i m) -> p i m", i=2)
                     for q in range(NQ)]
            for q in range(NQ):
                wt = const.tile([P, 2, 4 * H], f8, name=f"w8_{q}",
                                tag=f"w8_{q}")
                w_sb.append(wt)
            hT = []
            for t in range(T):
                ht = const.tile([P, NJ, b_loc], f8, name=f"hT{t}",
                                tag=f"hT{t}")
                hT.append(ht)
            # per-gate weight chunks in main-loop gate order (i, g, f, o),
            # interleaved with the first two timesteps' rhs halves
            for g in (G_I, G_G):
                for q in range(NQ):
                    gs = slice(g * H, (g + 1) * H)
                    nc.sync.dma_start(w_sb[q][:, :, gs], w8_ap[q][:, :, gs])
            nc.sync.dma_start(
                hT[0][:, :, 0:bgrp],
                hT_in.ap()[0].rearrange("p (k b) -> p k b", k=NJ)[:, :, 0:bgrp])
            for g in (G_F, G_O):
                for q in range(NQ):
                    gs = slice(g * H, (g + 1) * H)
                    nc.sync.dma_start(w_sb[q][:, :, gs], w8_ap[q][:, :, gs])
            nc.sync.dma_start(
                hT[1][:, :, 0:bgrp],
                hT_in.ap()[1].rearrange("p (k b) -> p k b", k=NJ)[:, :, 0:bgrp])
            for t in range(T):
                ap_t = hT_in.ap()[t].rearrange("p (k b) -> p k b", k=NJ)
                if t < 2:
                    nc.sync.dma_start(hT[t][:, :, bgrp:b_loc],
                                      ap_t[:, :, bgrp:b_loc])
                else:
                    nc.sync.dma_start(hT[t][:], ap_t)
            vd_sb = const.tile([P, NJ], bf16, name="vd_sb")
            nc.sync.dma_start(vd_sb[:], vd_in.ap())
            vh_sb = const.tile([P, CH], bf16, name="vh_sb")
            nc.sync.dma_start(vh_sb[:], vh_in.ap())
            h9_t = []
            for bt in range(NBT):
                h9 = const.tile([P, CH], bf16, name=f"h9_{bt}",
                                tag=f"h9_{bt}")
                nc.sync.dma_start(h9[:], h9_in.ap()[bt * P:(bt + 1) * P, :])
                h9_t.append(h9)
            yh = const.tile([P, NBT], f32, name="yh")

            def k2_matmuls(ps, gate, t, bg):
                """K=2 bias/y matmuls: 4-up tile_position pack, start=True
                clears the 4 banks (each j-region is exactly one bank)."""
                for j in range(NJ):
                    mi = gate * NJ + j
                    if y8:
                        nc.tensor.matmul(
                            ps[:, j * bgrp:(j + 1) * bgrp],
                            wb_sb[32 * j:32 * j + 1, :,
                                  mi * P:(mi + 1) * P],
                            yt_sb[32 * j:32 * j + 1, :,
                                  t * b_loc + bg * bgrp:
                                  t * b_loc + (bg + 1) * bgrp],
                            start=True, stop=False,
                            tile_position=(32 * j, 0),
                            perf_mode=DR,
                            skip_group_check=True)
                    else:
                        nc.tensor.matmul(
                            ps[:, j * bgrp:(j + 1) * bgrp],
                            wb_sb[32 * j:32 * j + 2, mi * P:(mi + 1) * P],
                            yt_sb[32 * j:32 * j + 2,
                                  t * b_loc + bg * bgrp:
                                  t * b_loc + (bg + 1) * bgrp],
                            start=True, stop=False,
                            tile_position=(32 * j, 0),
                            skip_group_check=True)

            def dr_matmuls(ps, gate, t, bg):
                """fp8 DoubleRow matmuls, K=256 each (two 128-ch regions)."""
                for j in range(NJ):
                    mi = gate * NJ + j
                    for q in range(NQ):
                        nc.tensor.matmul(
                            ps[:, j * bgrp:(j + 1) * bgrp],
                            w_sb[q][:, :, mi * P:(mi + 1) * P],
                            hT[t][:, 2 * q:2 * q + 2,
                                  bg * bgrp:(bg + 1) * bgrp],
                            start=False, stop=(q == NQ - 1),
                            perf_mode=DR,
                            skip_group_check=True)

            def gate_matmul(gate, t, bg):
                ps = psum.tile([P, GW], f32, name="ps_big", tag="ps")
                k2_matmuls(ps, gate, t, bg)
                dr_matmuls(ps, gate, t, bg)
                return ps

            # ---- main loop ----
            # Per bg: 30 gate tiles in order [I0 G0 I1 G1 F1 ... I9 G9 F9 O9]
            # processed in PAIRS: both tiles' K=2 groups issue together, then
            # both DR batches — halving the PE tile-config switch cost.
            y_d_t = []
            c_bg = []
            so_bg = []
            for bg in range(NBG):
                c_t = const.tile([P, GW], bf16 if c_bf16 else f32,
                                 name=f"c_{bg}", tag=f"c{bg}")
                so_t = fin.tile([P, GW], bf16, name="so", tag=f"so{bg}",
                                bufs=1)
                so_bg.append(so_t)

                gate_list = [(G_I, 0), (G_G, 0)]
                for t in range(1, T):
                    gate_list += [(G_I, t), (G_G, t), (G_F, t)]
                gate_list.append((G_O, T - 1))

                tiles = {}      # (gate, t) -> work tile with ACT output

                def emit_consumers(gate, t, c_t=c_t, tiles=tiles):
                    """DVE ops that become ready once (gate, t) is activated.

                    Called right after the ACT emission for (gate, t); FIFO
                    order on DVE preserves the c-chain sequence."""
                    if gate == G_G:
                        si = tiles.pop((G_I, t))
                        tg = tiles.pop((G_G, t))
                        if t == 0:
                            nc.vector.tensor_tensor(c_t[:], si[:], tg[:],
                                                    ALU.mult)
                        else:
                            m = work.tile([P, GW], bf16, name="m", tag="m")
                            nc.vector.tensor_tensor(m[:], si[:], tg[:],
                                                    ALU.mult)
                            tiles[("m", t)] = m
                    elif gate == G_F:
                        sf = tiles.pop((G_F, t))
                        m = tiles.pop(("m", t))
                        nc.vector.tensor_tensor(c_t[:], c_t[:], sf[:],
                                                ALU.mult)
                        nc.vector.tensor_tensor(c_t[:], c_t[:], m[:],
                                                ALU.add)

                if pair_gates:
                    for k in range(0, len(gate_list), 2):
                        pair = gate_list[k:k + 2]
                        pss = []
                        for gate, t in pair:
                            ps = psum.tile([P, GW], f32, name="ps_big",
                                           tag="ps")
                            k2_matmuls(ps, gate, t, bg)
                            pss.append(ps)
                        for (gate, t), ps in zip(pair, pss):
                            dr_matmuls(ps, gate, t, bg)
                        for (gate, t), ps in zip(pair, pss):
                            if gate == G_O:
                                nc.scalar.activation(so_t[:], ps[:],
                                                     AF.Sigmoid, scale=INV)
                            else:
                                func = AF.Tanh if gate == G_G else AF.Sigmoid
                                wt = work.tile([P, GW], bf16, name="act",
                                               tag="si" if gate == G_I else
                                               ("tg" if gate == G_G else
                                                "sf"))
                                nc.scalar.activation(wt[:], ps[:], func,
                                                     scale=INV)
                                tiles[(gate, t)] = wt
                            emit_consumers(gate, t)
                else:
                    for gate, t in gate_list:
                        ps = gate_matmul(gate, t, bg)
                        if gate == G_O:
                            # per-j chunks: lets the finalize chain start
                            # ~1.3us earlier at the tail
                            for j in range(NJ):
                                sl = slice(j * bgrp, (j + 1) * bgrp)
                                nc.scalar.activation(so_t[:, sl], ps[:, sl],
                                                     AF.Sigmoid, scale=INV)
                        else:
                            func = AF.Tanh if gate == G_G else AF.Sigmoid
                            wt = work.tile([P, GW], bf16, name="act",
                                           tag="si" if gate == G_I else
                                           ("tg" if gate == G_G else "sf"))
                            nc.scalar.activation(wt[:], ps[:], func,
                                                 scale=INV)
                            tiles[(gate, t)] = wt
                        emit_consumers(gate, t)

                c_bg.append(c_t)
                # y_h = h9 . v_h for this group's rows on DVE (has slack
                # while the other group computes)
                for u in range(NBT // NBG):
                    bt = bg * (NBT // NBG) + u
                    tmp = work.tile([P, CH], bf16, name="tmp9", tag="tmp9")
                    if use_ttr:
                        nc.vector.tensor_tensor_reduce(
                            tmp[:], h9_t[bt][:], vh_sb[:], 1.0, 0.0,
                            ALU.mult, ALU.add, yh[:, bt:bt + 1])
                    else:
                        nc.vector.tensor_tensor(tmp[:], h9_t[bt][:],
                                                vh_sb[:], ALU.mult)
                        nc.vector.tensor_reduce(yh[:, bt:bt + 1], tmp[:],
                                                mybir.AxisListType.X,
                                                ALU.add)

                # d = sigma(o) * tanh(c) per j-chunk now (ACT/DVE only, no
                # PSUM — bg0's part overlaps bg1's main loop); dots deferred
                d_bg = []
                for j in range(NJ):
                    sl = slice(j * bgrp, (j + 1) * bgrp)
                    tc9 = fin.tile([P, bgrp], bf16, name="tc9", tag="tc9",
                                   bufs=3)
                    nc.scalar.activation(tc9[:], c_t[:, sl], AF.Tanh)
                    d = fin.tile([P, bgrp], bf16, name="d", tag=f"d{bg}_{j}",
                                 bufs=1)
                    nc.vector.tensor_tensor(d[:], so_t[:, sl], tc9[:],
                                            ALU.mult)
                    d_bg.append(d)
                y_d_t.append(d_bg)

            # ---- final dots (after all gate psum tiles -> no rotation
            # stalls) ----
            for bg in range(NBG):
                ps_y = psum.tile([1, bgrp], f32, name="ps_y", tag="ps")
                for j in range(NJ):
                    nc.tensor.matmul(ps_y[:], vd_sb[:, j:j + 1],
                                     y_d_t[bg][j][:],
                                     start=(j == 0), stop=(j == NJ - 1))
                y_d = fin.tile([1, bgrp], f32, name="y_d", tag=f"y_d{bg}",
                               bufs=1)
                nc.vector.tensor_copy(y_d[:], ps_y[:])
                nc.sync.dma_start(outd.ap()[bg * bgrp:(bg + 1) * bgrp],
                                  y_d[:])

            nc.sync.dma_start(outh.ap(), yh[:])

    nc.compile()
    return nc


def _host_prep(inputs):
    W_hh = np.asarray(inputs["W_hh"], np.float32)
    W_ih = np.asarray(inputs["W_ih"], np.float32)
    b = (np.asarray(inputs["b_ih"], np.float32)
         + np.asarray(inputs["b_hh"], np.float32))          # [2048]
    fc1_w = np.asarray(inputs["fc1_w"], np.float32)
    fc2_w = np.asarray(inputs["fc2_w"], np.float32)
    v = (fc2_w @ fc1_w)[0]                                   # [1024]
    c0 = float(np.asarray(inputs["fc1_b"], np.float32) @ fc2_w[0]
               + np.asarray(inputs["fc2_b"], np.float32)[0])

    NJ = H // P
    # w8[q, p, i, m] = fp8(W_hh[m, 128*(2q+i) + p] * S)
    W8T = (W_hh * S).astype(F8).T                            # [512, 2048]
    w8 = np.ascontiguousarray(
        W8T.reshape(2, 2, P, 4 * H).transpose(0, 2, 1, 3)
    ).reshape(2, P, 2 * 4 * H)
    wb_f32 = np.stack([W_ih[:, 0] * S, b * S])               # [2, 2048]
    if Y8:
        wih_b = np.ascontiguousarray(wb_f32.astype(F8))[None]
    else:
        wih_b = np.ascontiguousarray(wb_f32.astype(BF16))
    v_d = np.ascontiguousarray(v[:H].reshape(NJ, P).T.copy())       # [128,4]
    v_h = np.ascontiguousarray(np.tile(v[H:][None, :], (P, 1)))     # [128,512]
    return w8, wih_b, v_d.astype(BF16), v_h.astype(BF16), c0


def _install_ntff_shim():
    """Best-effort: recreate antenv.axon_hooks so trace=True can profile."""
    import sys as _sys
    import types as _types
    try:
        import antenv.axon_hooks  # noqa: F401
        return
    except ImportError:
        pass
    try:
        import antenv
        from trn_agent_boot.trn_boot import _ntff_profile_via_ctypes
        hook = _ntff_profile_via_ctypes("/opt/axon/libaxon_pjrt.so")
        mod = _types.ModuleType("antenv.axon_hooks")
        _state = {"hook": hook}
        mod.set_axon_ntff_profile_hook = lambda hk: _state.__setitem__("hook", hk)
        mod.get_axon_ntff_profile_hook = lambda: _state["hook"]
        _sys.modules["antenv.axon_hooks"] = mod
        antenv.axon_hooks = mod
    except Exception:
        pass


def make_in_maps(inputs):
    w8, wih_b, v_d, v_h, c0 = _host_prep(inputs)
    h = np.asarray(inputs["h"], np.float32)
    y = np.asarray(inputs["y_seq"], np.float32)
    NJ = H // P
    in_maps = []
    for c in range(N_CORES):
        sl = slice(c * B_LOC, (c + 1) * B_LOC)
        h_sl = h[sl]                                         # [1024, 10, 512]
        h8 = h_sl.astype(F8)
        # hT8[t, p, k*1024 + b] = fp8(h[b, t, 128k + p])
        hT8 = np.ascontiguousarray(
            h8.transpose(1, 2, 0)                            # [T, CH, b_loc]
            .reshape(T, NJ, P, B_LOC)
            .transpose(0, 2, 1, 3)                           # [T, P, NJ, b]
        ).reshape(T, P, NJ * B_LOC)
        if Y8:
            yt = np.empty((1, 2, T * B_LOC), F8)
            yt[0, 0] = y[sl].T.astype(F8).reshape(-1)
            yt[0, 1] = np.ones(T * B_LOC, F8)
            wb_key = "wb8"
        else:
            yt = np.empty((2, T, B_LOC), BF16)
            yt[0] = y[sl].T.astype(BF16)
            yt[1] = np.ones((T, B_LOC), BF16)
            wb_key = "wih_b"
        in_maps.append({
            "hT8": hT8,
            "h9": np.ascontiguousarray(h_sl[:, T - 1, :].astype(BF16)),
            "yt_aug": yt,
            "w8": w8, wb_key: wih_b,
            "v_d": v_d, "v_h": v_h,
        })
    return in_maps, c0


def run(inputs, trace=False):
    key = "full"
    if key not in _compiled:
        _compiled[key] = build_nc()
    nc = _compiled[key]

    if trace:
        _install_ntff_shim()

    in_maps, c0 = make_in_maps(inputs)
    res = run_bass_kernel_spmd(nc, in_maps, core_ids=list(range(N_CORES)),
                               trace=trace)
    outs = []
    for c in range(N_CORES):
        r = res.results[c]
        y_core = (r["out_d"] + r["out_h"].T.reshape(-1) + c0)
        outs.append(y_core.astype(np.float32))
    return np.concatenate(outs)[:, None], res


def kernel(**inputs):
    out, _ = run(inputs, trace=False)
    return out


# revision 35
# speedup vs baseline: 1.1008x; 1.0005x over previous
"""Trainium2 Bass kernel for nn_AttnDecoder (B=8192, T=10, CH=H=512).

Math notes (verified against the jax reference in fp32 to ~3e-6):
  - The attention block is dead code: softmax over a size-1 axis == 1, so
    h1 == ht and attn1/2/3 never affect the output.
  - The LSTM hidden state d never feeds back into the gates (only the cell
    state c does, elementwise), so the only sequential part is
        c_t = sigmoid(f_t) * c_{t-1} + sigmoid(i_t) * tanh(g_t)
    a cheap elementwise recurrence over T=10.
  - o-gate is only needed at t = T-1.
  - fc2(fc1(z)) with no nonlinearity folds into a single vector:
        y = d . v[:H] + h9 . v[H:] + c0,   v = (fc2_w @ fc1_w)^T.

Sharding: batch-parallel over 8 cores (1024 rows each), weights replicated.

Implementation (v2, fp8 DoubleRow):
  - h is pre-cast to fp8-e4m3 and pre-transposed on the host into
    hT8[t, p, k*1024+b] = fp8(h[b, t, 128k+p]); whole-tensor rel-err budget
    allows it (fp8 pipeline sims at 8.3e-3 vs the 2e-2 gate).
  - Gate matmuls run in fp8 DoubleRow mode: each instruction contracts
    K=256 (two 128-channel regions, lhsT/rhs APs shaped [128, 2, n]),
    halving PE streaming time vs bf16.
  - Weights are pre-scaled by S=16 on the host (better fp8 mantissa use);
    the activation instruction compensates with scale=1/S for free.
  - y_t / bias enter via a K=2 bf16 matmul (rows [y;1] x [w_ih*S; b*S]),
    4-up tile_position-packed, accumulating into the same PSUM banks.
  - Cell state c is bf16 (DVE 2x mode); sim shows no accuracy change.
  - Everything is resident in SBUF up-front (~120KB/partition): no SWDGE,
    no DRAM staging, no transpose DMAs, no gpsimd.
  - Final: d = sigma(o)*tanh(c); y_d via fp32 PE dot with v[:H];
    y_h = h9 . v[H:] fused mult+reduce on DVE; partials summed on host.
"""

import numpy as np
import ml_dtypes

import concourse.bass as bass
import concourse.tile as tile
from concourse import bacc, mybir
from concourse.bass_utils import run_bass_kernel_spmd

BF16 = ml_dtypes.bfloat16
F8 = ml_dtypes.float8_e4m3

B, T, CH, H = 8192, 10, 512, 512
N_CORES = 8
B_LOC = B // N_CORES            # 1024 batch rows per core
P = 128
S = 16.0                        # fp8 weight pre-scale
Y8 = False                      # y/bias matmul operands in fp8 DR form

_compiled = {}


def build_nc(b_loc=B_LOC, bgrp=512, psum_bufs=2, work_bufs=3,
             use_ttr=False, c_bf16=True, use_dr=True, y8=Y8,
             pair_gates=False):
    NBG = b_loc // bgrp         # batch groups
    NJ = H // P                 # 4 hid chunks
    NQ = CH // (2 * P)          # 2 DoubleRow K-chunks (256 channels each)
    NBT = b_loc // P            # batch tiles for the h9 dot
    GW = NJ * bgrp              # big-tile width (one gate, all hid chunks)
    f32 = mybir.dt.float32
    bf16 = mybir.dt.bfloat16
    f8 = mybir.dt.float8e4
    AF = mybir.ActivationFunctionType
    ALU = mybir.AluOpType
    DR = mybir.MatmulPerfMode.DoubleRow

    nc = bacc.Bacc("TRN2", target_bir_lowering=False, debug=False,
                   num_devices=N_CORES)

    # hT8[t, p, k*b_loc + b] = fp8(h[b, t, 128k + p])
    hT_in = nc.dram_tensor("hT8", [T, P, NJ * b_loc], f8, kind="ExternalInput")
    h9_in = nc.dram_tensor("h9", [b_loc, CH], bf16, kind="ExternalInput")
    # per t: rhs rows [y_t ; ones] for the K=2 bias/y matmul
    if y8:
        yt_in = nc.dram_tensor("yt_aug", [1, 2, T * b_loc], f8,
                               kind="ExternalInput")
        wb8_in = nc.dram_tensor("wb8", [1, 2, 4 * H], f8,
                                kind="ExternalInput")
    else:
        yt_in = nc.dram_tensor("yt_aug", [2, T, b_loc], bf16,
                               kind="ExternalInput")
    # w8[q, p, i*2048 + m] = fp8(W_hh[m, 128*(2q+i) + p] * S)
    w8_in = nc.dram_tensor("w8", [NQ, P, 2 * 4 * H], f8, kind="ExternalInput")
    if not y8:
        # rows: [w_ih * S ; (b_ih + b_hh) * S]
        wb_in = nc.dram_tensor("wih_b", [2, 4 * H], bf16,
                               kind="ExternalInput")
    vd_in = nc.dram_tensor("v_d", [P, NJ], bf16, kind="ExternalInput")
    vh_in = nc.dram_tensor("v_h", [P, CH], bf16, kind="ExternalInput")
    outd = nc.dram_tensor("out_d", [b_loc], f32, kind="ExternalOutput")
    outh = nc.dram_tensor("out_h", [P, NBT], f32, kind="ExternalOutput")

    G_I, G_F, G_G, G_O = 0, 1, 2, 3     # gate blocks in the 2048 W columns
    INV = 1.0 / S

    with tile.TileContext(nc) as tc:
        with (
            tc.tile_pool(name="const", bufs=1) as const,
            tc.tile_pool(name="work", bufs=work_bufs) as work,
            tc.tile_pool(name="fin", bufs=2) as fin,
            tc.tile_pool(name="psum", bufs=psum_bufs, space="PSUM") as psum,
        ):
            # ---- weights / constants into SBUF (small K=2 operands first;
            # fp8 weights and hT[0..1] in fine-grained gate/bgrp chunks so
            # the first matmuls' prerequisites land early) ----
            nrep = NJ
            if y8:
                wb_sb = const.tile([(nrep - 1) * 32 + 1, 2, 4 * H], f8,
                                   name="wb_sb")
                yt_sb = const.tile([(nrep - 1) * 32 + 1, 2, T * b_loc], f8,
                                   name="yt_sb")
                for r in range(nrep):
                    nc.sync.dma_start(wb_sb[32 * r:32 * r + 1, :, :],
                                      wb8_in.ap())
                    nc.sync.dma_start(yt_sb[32 * r:32 * r + 1, :, :],
                                      yt_in.ap())
            else:
                wb_sb = const.tile([(nrep - 1) * 32 + 2, 4 * H], bf16,
                                   name="wb_sb")
                yt_sb = const.tile([(nrep - 1) * 32 + 2, T * b_loc], bf16,
                                   name="yt_sb")
                for r in range(nrep):
                    nc.sync.dma_start(wb_sb[32 * r:32 * r + 2, :], wb_in.ap())
                    nc.sync.dma_start(
                        yt_sb[32 * r:32 * r + 2, :],
                        yt_in.ap().rearrange("r t b -> r (t b)"))
            w_sb = []
            w8_ap = [w8_in.ap()[q].rearrange("p (i m) -> p i m", i=2)
                     for q in range(NQ)]
            for q in range(NQ):
                wt = const.tile([P, 2, 4 * H], f8, name=f"w8_{q}",
                                tag=f"w8_{q}")
                w_sb.append(wt)
            hT = []
            for t in range(T):
                ht = const.tile([P, NJ, b_loc], f8, name=f"hT{t}",
                                tag=f"hT{t}")
                hT.append(ht)
            # per-gate weight chunks in main-loop gate order (i, g, f, o),
            # interleaved with the first two timesteps' rhs halves
            for g in (G_I, G_G):
                for q in range(NQ):
                    gs = slice(g * H, (g + 1) * H)
                    nc.sync.dma_start(w_sb[q][:, :, gs], w8_ap[q][:, :, gs])
            nc.sync.dma_start(
                hT[0][:, :, 0:bgrp],
                hT_in.ap()[0].rearrange("p (k b) -> p k b", k=NJ)[:, :, 0:bgrp])
            for g in (G_F, G_O):
                for q in range(NQ):
                    gs = slice(g * H, (g + 1) * H)
                    nc.sync.dma_start(w_sb[q][:, :, gs], w8_ap[q][:, :, gs])
            nc.sync.dma_start(
                hT[1][:, :, 0:bgrp],
                hT_in.ap()[1].rearrange("p (k b) -> p k b", k=NJ)[:, :, 0:bgrp])
            for t in range(T):
                ap_t = hT_in.ap()[t].rearrange("p (k b) -> p k b", k=NJ)
                if t < 2:
                    nc.sync.dma_start(hT[t][:, :, bgrp:b_loc],
                                      ap_t[:, :, bgrp:b_loc])
                else:
                    nc.sync.dma_start(hT[t][:], ap_t)
            vd_sb = const.tile([P, NJ], bf16, name="vd_sb")
            nc.sync.dma_start(vd_sb[:], vd_in.ap())
            vh_sb = const.tile([P, CH], bf16, name="vh_sb")
            nc.sync.dma_start(vh_sb[:], vh_in.ap())
            h9_t = []
            for bt in range(NBT):
                h9 = const.tile([P, CH], bf16, name=f"h9_{bt}",
                                tag=f"h9_{bt}")
                nc.sync.dma_start(h9[:], h9_in.ap()[bt * P:(bt + 1) * P, :])
                h9_t.append(h9)
            yh = const.tile([P, NBT], f32, name="yh")

            def k2_matmuls(ps, gate, t, bg):
                """K=2 bias/y matmuls: 4-up tile_position pack, start=True
                clears the 4 banks (each j-region is exactly one bank)."""
                for j in range(NJ):
                    mi = gate * NJ + j
                    if y8:
                        nc.tensor.matmul(
                            ps[:, j * bgrp:(j + 1) * bgrp],
                            wb_sb[32 * j:32 * j + 1, :,
                                  mi * P:(mi + 1) * P],
                            yt_sb[32 * j:32 * j + 1, :,
                                  t * b_loc + bg * bgrp:
                                  t * b_loc + (bg + 1) * bgrp],
                            start=True, stop=False,
                            tile_position=(32 * j, 0),
                            perf_mode=DR,
                            skip_group_check=True)
                    else:
                        nc.tensor.matmul(
                            ps[:, j * bgrp:(j + 1) * bgrp],
                            wb_sb[32 * j:32 * j + 2, mi * P:(mi + 1) * P],
                            yt_sb[32 * j:32 * j + 2,
                                  t * b_loc + bg * bgrp:
                                  t * b_loc + (bg + 1) * bgrp],
                            start=True, stop=False,
                            tile_position=(32 * j, 0),
                            skip_group_check=True)

            def dr_matmuls(ps, gate, t, bg):
                """fp8 DoubleRow matmuls, K=256 each (two 128-ch regions)."""
                for j in range(NJ):
                    mi = gate * NJ + j
                    for q in range(NQ):
                        nc.tensor.matmul(
                            ps[:, j * bgrp:(j + 1) * bgrp],
                            w_sb[q][:, :, mi * P:(mi + 1) * P],
                            hT[t][:, 2 * q:2 * q + 2,
                                  bg * bgrp:(bg + 1) * bgrp],
                            start=False, stop=(q == NQ - 1),
                            perf_mode=DR,
                            skip_group_check=True)

            def gate_matmul(gate, t, bg):
                ps = psum.tile([P, GW], f32, name="ps_big", tag="ps")
                k2_matmuls(ps, gate, t, bg)
                dr_matmuls(ps, gate, t, bg)
                return ps

            # ---- main loop ----
            # Per bg: 30 gate tiles in order [I0 G0 I1 G1 F1 ... I9 G9 F9 O9]
            # processed in PAIRS: both tiles' K=2 groups issue together, then
            # both DR batches — halving the PE tile-config switch cost.
            y_d_t = []
            c_bg = []
            so_bg = []
            for bg in range(NBG):
                c_t = const.tile([P, GW], bf16 if c_bf16 else f32,
                                 name=f"c_{bg}", tag=f"c{bg}")
                so_t = fin.tile([P, GW], bf16, name="so", tag=f"so{bg}",
                                bufs=1)
                so_bg.append(so_t)

                gate_list = [(G_I, 0), (G_G, 0)]
                for t in range(1, T):
                    gate_list += [(G_I, t), (G_G, t), (G_F, t)]
                gate_list.append((G_O, T - 1))

                tiles = {}      # (gate, t) -> work tile with ACT output

                def emit_consumers(gate, t, c_t=c_t, tiles=tiles):
                    """DVE ops that become ready once (gate, t) is activated.

                    Called right after the ACT emission for (gate, t); FIFO
                    order on DVE preserves the c-chain sequence."""
                    if gate == G_G:
                        si = tiles.pop((G_I, t))
                        tg = tiles.pop((G_G, t))
                        if t == 0:
                            nc.vector.tensor_tensor(c_t[:], si[:], tg[:],
                                                    ALU.mult)
                        else:
                            m = work.tile([P, GW], bf16, name="m", tag="m")
                            nc.vector.tensor_tensor(m[:], si[:], tg[:],
                                                    ALU.mult)
                            tiles[("m", t)] = m
                    elif gate == G_F:
                        sf = tiles.pop((G_F, t))
                        m = tiles.pop(("m", t))
                        nc.vector.tensor_tensor(c_t[:], c_t[:], sf[:],
                                                ALU.mult)
                        nc.vector.tensor_tensor(c_t[:], c_t[:], m[:],
                                                ALU.add)

                if pair_gates:
                    for k in range(0, len(gate_list), 2):
                        pair = gate_list[k:k + 2]
                        pss = []
                        for gate, t in pair:
                            ps = psum.tile([P, GW], f32, name="ps_big",
                                           tag="ps")
                            k2_matmuls(ps, gate, t, bg)
                            pss.append(ps)
                        for (gate, t), ps in zip(pair, pss):
                            dr_matmuls(ps, gate, t, bg)
                        for (gate, t), ps in zip(pair, pss):
                            if gate == G_O:
                                nc.scalar.activation(so_t[:], ps[:],
                                                     AF.Sigmoid, scale=INV)
                            else:
                                func = AF.Tanh if gate == G_G else AF.Sigmoid
                                wt = work.tile([P, GW], bf16, name="act",
                                               tag="si" if gate == G_I else
                                               ("tg" if gate == G_G else
                                                "sf"))
                                nc.scalar.activation(wt[:], ps[:], func,
                                                     scale=INV)
                                tiles[(gate, t)] = wt
                            emit_consumers(gate, t)
                else:
                    for gate, t in gate_list:
                        ps = gate_matmul(gate, t, bg)
                        if gate == G_O:
                            # per-j chunks: lets the finalize chain start
                            # ~1.3us earlier at the tail
                            for j in range(NJ):
                                sl = slice(j * bgrp, (j + 1) * bgrp)
                                nc.scalar.activation(so_t[:, sl], ps[:, sl],
                                                     AF.Sigmoid, scale=INV)
                        else:
                            func = AF.Tanh if gate == G_G else AF.Sigmoid
                            wt = work.tile([P, GW], bf16, name="act",
                                           tag="si" if gate == G_I else
                                           ("tg" if gate == G_G else "sf"))
                            nc.scalar.activation(wt[:], ps[:], func,
                                                 scale=INV)
                            tiles[(gate, t)] = wt
                        emit_consumers(gate, t)

                c_bg.append(c_t)
                # y_h = h9 . v_h: all 8 batch tiles emitted after bg0's loop
                # (DVE has slack while bg1 computes; outh DMA leaves the
                # tail)
                if bg == 0:
                    for bt in range(NBT):
                        tmp = work.tile([P, CH], bf16, name="tmp9",
                                        tag="tmp9")
                        nc.vector.tensor_tensor(tmp[:], h9_t[bt][:],
                                                vh_sb[:], ALU.mult)
                        nc.vector.tensor_reduce(yh[:, bt:bt + 1], tmp[:],
                                                mybir.AxisListType.X,
                                                ALU.add)
                    nc.sync.dma_start(outh.ap(), yh[:])

                # d = sigma(o) * tanh(c) per j-chunk now (ACT/DVE only, no
                # PSUM — bg0's part overlaps bg1's main loop); dots deferred
                d_bg = []
                for j in range(NJ):
                    sl = slice(j * bgrp, (j + 1) * bgrp)
                    tc9 = fin.tile([P, bgrp], bf16, name="tc9", tag="tc9",
                                   bufs=3)
                    nc.scalar.activation(tc9[:], c_t[:, sl], AF.Tanh)
                    d = fin.tile([P, bgrp], bf16, name="d", tag=f"d{bg}_{j}",
                                 bufs=1)
                    nc.vector.tensor_tensor(d[:], so_t[:, sl], tc9[:],
                                            ALU.mult)
                    d_bg.append(d)
                y_d_t.append(d_bg)

            # ---- final dots (after all gate psum tiles -> no rotation
            # stalls) ----
            for bg in range(NBG):
                ps_y = psum.tile([1, bgrp], f32, name="ps_y", tag="ps")
                for j in range(NJ):
                    nc.tensor.matmul(ps_y[:], vd_sb[:, j:j + 1],
                                     y_d_t[bg][j][:],
                                     start=(j == 0), stop=(j == NJ - 1))
                y_d = fin.tile([1, bgrp], f32, name="y_d", tag=f"y_d{bg}",
                               bufs=1)
                nc.vector.tensor_copy(y_d[:], ps_y[:])
                nc.sync.dma_start(outd.ap()[bg * bgrp:(bg + 1) * bgrp],
                                  y_d[:])


    nc.compile()
    return nc


def _host_prep(inputs):
    W_hh = np.asarray(inputs["W_hh"], np.float32)
    W_ih = np.asarray(inputs["W_ih"], np.float32)
    b = (np.asarray(inputs["b_ih"], np.float32)
         + np.asarray(inputs["b_hh"], np.float32))          # [2048]
    fc1_w = np.asarray(inputs["fc1_w"], np.float32)
    fc2_w = np.asarray(inputs["fc2_w"], np.float32)
    v = (fc2_w @ fc1_w)[0]                                   # [1024]
    c0 = float(np.asarray(inputs["fc1_b"], np.float32) @ fc2_w[0]
               + np.asarray(inputs["fc2_b"], np.float32)[0])

    NJ = H // P
    # w8[q, p, i, m] = fp8(W_hh[m, 128*(2q+i) + p] * S)
    W8T = (W_hh * S).astype(F8).T                            # [512, 2048]
    w8 = np.ascontiguousarray(
        W8T.reshape(2, 2, P, 4 * H).transpose(0, 2, 1, 3)
    ).reshape(2, P, 2 * 4 * H)
    wb_f32 = np.stack([W_ih[:, 0] * S, b * S])               # [2, 2048]
    if Y8:
        wih_b = np.ascontiguousarray(wb_f32.astype(F8))[None]
    else:
        wih_b = np.ascontiguousarray(wb_f32.astype(BF16))
    v_d = np.ascontiguousarray(v[:H].reshape(NJ, P).T.copy())       # [128,4]
    v_h = np.ascontiguousarray(np.tile(v[H:][None, :], (P, 1)))     # [128,512]
    return w8, wih_b, v_d.astype(BF16), v_h.astype(BF16), c0


def _install_ntff_shim():
    """Best-effort: recreate antenv.axon_hooks so trace=True can profile."""
    import sys as _sys
    import types as _types
    try:
        import antenv.axon_hooks  # noqa: F401
        return
    except ImportError:
        pass
    try:
        import antenv
        from trn_agent_boot.trn_boot import _ntff_profile_via_ctypes
        hook = _ntff_profile_via_ctypes("/opt/axon/libaxon_pjrt.so")
        mod = _types.ModuleType("antenv.axon_hooks")
        _state = {"hook": hook}
        mod.set_axon_ntff_profile_hook = lambda hk: _state.__setitem__("hook", hk)
        mod.get_axon_ntff_profile_hook = lambda: _state["hook"]
        _sys.modules["antenv.axon_hooks"] = mod
        antenv.axon_hooks = mod
    except Exception:
        pass


def make_in_maps(inputs):
    w8, wih_b, v_d, v_h, c0 = _host_prep(inputs)
    h = np.asarray(inputs["h"], np.float32)
    y = np.asarray(inputs["y_seq"], np.float32)
    NJ = H // P
    in_maps = []
    for c in range(N_CORES):
        sl = slice(c * B_LOC, (c + 1) * B_LOC)
        h_sl = h[sl]                                         # [1024, 10, 512]
        h8 = h_sl.astype(F8)
        # hT8[t, p, k*1024 + b] = fp8(h[b, t, 128k + p])
        hT8 = np.ascontiguousarray(
            h8.transpose(1, 2, 0)                            # [T, CH, b_loc]
            .reshape(T, NJ, P, B_LOC)
            .transpose(0, 2, 1, 3)                           # [T, P, NJ, b]
        ).reshape(T, P, NJ * B_LOC)
        if Y8:
            yt = np.empty((1, 2, T * B_LOC), F8)
            yt[0, 0] = y[sl].T.astype(F8).reshape(-1)
            yt[0, 1] = np.ones(T * B_LOC, F8)
            wb_key = "wb8"
        else:
            yt = np.empty((2, T, B_LOC), BF16)
            yt[0] = y[sl].T.astype(BF16)
            yt[1] = np.ones((T, B_LOC), BF16)
            wb_key = "wih_b"
        in_maps.append({
            "hT8": hT8,
            "h9": np.ascontiguousarray(h_sl[:, T - 1, :].astype(BF16)),
            "yt_aug": yt,
            "w8": w8, wb_key: wih_b,
            "v_d": v_d, "v_h": v_h,
        })
    return in_maps, c0


def run(inputs, trace=False, retries=2):
    key = "full"
    if key not in _compiled:
        _compiled[key] = build_nc()
    nc = _compiled[key]

    if trace:
        _install_ntff_shim()

    in_maps, c0 = make_in_maps(inputs)
    res = None
    for attempt in range(retries + 1):
        try:
            res = run_bass_kernel_spmd(nc, in_maps,
                                       core_ids=list(range(N_CORES)),
                                       trace=trace)
            break
        except Exception:
            # the device sporadically wedges (NRT_EXEC_UNIT_UNRECOVERABLE);
            # a simple retry after poking the runtime usually recovers it
            if attempt == retries:
                raise
            import time as _time
            _time.sleep(2.0)
    outs = []
    for c in range(N_CORES):
        r = res.results[c]
        y_core = (r["out_d"] + r["out_h"].T.reshape(-1) + c0)
        outs.append(y_core.astype(np.float32))
    return np.concatenate(outs)[:, None], res


def kernel(**inputs):
    out, _ = run(inputs, trace=False)
    return out


# revision 36
# speedup vs baseline: 1.1047x; 1.0035x over previous
"""Trainium2 Bass kernel for nn_AttnDecoder (B=8192, T=10, CH=H=512).

Math notes (verified against the jax reference in fp32 to ~3e-6):
  - The attention block is dead code: softmax over a size-1 axis == 1, so
    h1 == ht and attn1/2/3 never affect the output.
  - The LSTM hidden state d never feeds back into the gates (only the cell
    state c does, elementwise), so the only sequential part is
        c_t = sigmoid(f_t) * c_{t-1} + sigmoid(i_t) * tanh(g_t)
    a cheap elementwise recurrence over T=10.
  - o-gate is only needed at t = T-1.
  - fc2(fc1(z)) with no nonlinearity folds into a single vector:
        y = d . v[:H] + h9 . v[H:] + c0,   v = (fc2_w @ fc1_w)^T.

Sharding: batch-parallel over 8 cores (1024 rows each), weights replicated.

Implementation (v2, fp8 DoubleRow; ~167us vs the 309us bf16 baseline):
  - h is pre-cast to fp8-e4m3 and pre-transposed on the host into
    hT8[t, p, k*1024+b] = fp8(h[b, t, 128k+p]); whole-tensor rel-err budget
    allows it (measures 8.6e-3 on HW vs the 2e-2 gate).
  - Gate matmuls run in fp8 DoubleRow mode: each instruction contracts
    K=256 (two 128-channel regions, lhsT/rhs APs shaped [128, 2, n]),
    halving PE streaming time vs bf16; steady-state MMs hit the 216ns
    (512 cols @ 2.4GHz) floor.
  - Weights are pre-scaled by S=16 on the host (better fp8 mantissa use);
    the activation instruction compensates with scale=1/S for free.
  - y_t / bias enter via a K=2 bf16 matmul (rows [y;1] x [w_ih*S; b*S]),
    4-up tile_position-packed (concurrent sub-array strips), accumulating
    into the same PSUM banks with start=True.
  - PSUM: two 4-bank [128, 2048] tiles rotate; ACT drains one (2.0us)
    while the PE fills the other (2.2us) — balanced, so finer tilings or
    gate-pairing lose (measured).
  - Cell state c is bf16 (DVE 2x mode); finalize path (so/tc9/d/v) bf16.
  - Everything is resident in SBUF up-front (~120KB/partition): no SWDGE,
    no DRAM staging, no transpose DMAs, no gpsimd. DMAs ordered so the
    first gates' operands land first; all loads on the sync HWDGE ring
    (issuing bulk from nc.scalar stalls the ACT FIFO — measured +18us).
  - Final: d = sigma(o)*tanh(c) per j-chunk; y_d via bf16 PE dot with
    v[:H] deferred past all gate tiles (no psum-rotation stall);
    y_h = h9 . v[H:] on DVE mid-run; partials summed on host.
  - Known traps hit: tensor_tensor_reduce crashes the exec unit (use
    mult+reduce); interleaving perf-modes inside one accumulation group
    costs ~35us in PE drains.
"""

import numpy as np
import ml_dtypes

import concourse.bass as bass
import concourse.tile as tile
from concourse import bacc, mybir
from concourse.bass_utils import run_bass_kernel_spmd

BF16 = ml_dtypes.bfloat16
F8 = ml_dtypes.float8_e4m3

B, T, CH, H = 8192, 10, 512, 512
N_CORES = 8
B_LOC = B // N_CORES            # 1024 batch rows per core
P = 128
S = 16.0                        # fp8 weight pre-scale
Y8 = False                      # y/bias matmul operands in fp8 DR form

_compiled = {}


def build_nc(b_loc=B_LOC, bgrp=512, psum_bufs=2, work_bufs=3,
             use_ttr=False, c_bf16=True, use_dr=True, y8=Y8,
             pair_gates=False):
    NBG = b_loc // bgrp         # batch groups
    NJ = H // P                 # 4 hid chunks
    NQ = CH // (2 * P)          # 2 DoubleRow K-chunks (256 channels each)
    NBT = b_loc // P            # batch tiles for the h9 dot
    GW = NJ * bgrp              # big-tile width (one gate, all hid chunks)
    f32 = mybir.dt.float32
    bf16 = mybir.dt.bfloat16
    f8 = mybir.dt.float8e4
    AF = mybir.ActivationFunctionType
    ALU = mybir.AluOpType
    DR = mybir.MatmulPerfMode.DoubleRow

    nc = bacc.Bacc("TRN2", target_bir_lowering=False, debug=False,
                   num_devices=N_CORES)

    # hT8[t, p, k*b_loc + b] = fp8(h[b, t, 128k + p])
    hT_in = nc.dram_tensor("hT8", [T, P, NJ * b_loc], f8, kind="ExternalInput")
    h9_in = nc.dram_tensor("h9", [b_loc, CH], bf16, kind="ExternalInput")
    # per t: rhs rows [y_t ; ones] for the K=2 bias/y matmul
    if y8:
        yt_in = nc.dram_tensor("yt_aug", [1, 2, T * b_loc], f8,
                               kind="ExternalInput")
        wb8_in = nc.dram_tensor("wb8", [1, 2, 4 * H], f8,
                                kind="ExternalInput")
    else:
        yt_in = nc.dram_tensor("yt_aug", [2, T, b_loc], bf16,
                               kind="ExternalInput")
    # w8[q, p, i*2048 + m] = fp8(W_hh[m, 128*(2q+i) + p] * S)
    w8_in = nc.dram_tensor("w8", [NQ, P, 2 * 4 * H], f8, kind="ExternalInput")
    if not y8:
        # rows: [w_ih * S ; (b_ih + b_hh) * S]
        wb_in = nc.dram_tensor("wih_b", [2, 4 * H], bf16,
                               kind="ExternalInput")
    vd_in = nc.dram_tensor("v_d", [P, NJ], bf16, kind="ExternalInput")
    vh_in = nc.dram_tensor("v_h", [P, CH], bf16, kind="ExternalInput")
    outd = nc.dram_tensor("out_d", [b_loc], f32, kind="ExternalOutput")
    outh = nc.dram_tensor("out_h", [P, NBT], f32, kind="ExternalOutput")

    G_I, G_F, G_G, G_O = 0, 1, 2, 3     # gate blocks in the 2048 W columns
    INV = 1.0 / S

    with tile.TileContext(nc) as tc:
        with (
            tc.tile_pool(name="const", bufs=1) as const,
            tc.tile_pool(name="work", bufs=work_bufs) as work,
            tc.tile_pool(name="fin", bufs=2) as fin,
            tc.tile_pool(name="psum", bufs=psum_bufs, space="PSUM") as psum,
        ):
            # ---- weights / constants into SBUF (small K=2 operands first;
            # fp8 weights and hT[0..1] in fine-grained gate/bgrp chunks so
            # the first matmuls' prerequisites land early) ----
            nrep = NJ
            if y8:
                wb_sb = const.tile([(nrep - 1) * 32 + 1, 2, 4 * H], f8,
                                   name="wb_sb")
                yt_sb = const.tile([(nrep - 1) * 32 + 1, 2, T * b_loc], f8,
                                   name="yt_sb")
                for r in range(nrep):
                    nc.sync.dma_start(wb_sb[32 * r:32 * r + 1, :, :],
                                      wb8_in.ap())
                    nc.sync.dma_start(yt_sb[32 * r:32 * r + 1, :, :],
                                      yt_in.ap())
            else:
                wb_sb = const.tile([(nrep - 1) * 32 + 2, 4 * H], bf16,
                                   name="wb_sb")
                yt_sb = const.tile([(nrep - 1) * 32 + 2, T * b_loc], bf16,
                                   name="yt_sb")
                for r in range(nrep):
                    nc.sync.dma_start(wb_sb[32 * r:32 * r + 2, :], wb_in.ap())
                    nc.sync.dma_start(
                        yt_sb[32 * r:32 * r + 2, :],
                        yt_in.ap().rearrange("r t b -> r (t b)"))
            w_sb = []
            w8_ap = [w8_in.ap()[q].rearrange("p (i m) -> p i m", i=2)
                     for q in range(NQ)]
            for q in range(NQ):
                wt = const.tile([P, 2, 4 * H], f8, name=f"w8_{q}",
                                tag=f"w8_{q}")
                w_sb.append(wt)
            hT = []
            for t in range(T):
                ht = const.tile([P, NJ, b_loc], f8, name=f"hT{t}",
                                tag=f"hT{t}")
                hT.append(ht)
            # per-gate weight chunks in main-loop gate order (i, g, f, o),
            # interleaved with the first two timesteps' rhs halves
            for g in (G_I, G_G):
                for q in range(NQ):
                    gs = slice(g * H, (g + 1) * H)
                    nc.sync.dma_start(w_sb[q][:, :, gs], w8_ap[q][:, :, gs])
            nc.sync.dma_start(
                hT[0][:, :, 0:bgrp],
                hT_in.ap()[0].rearrange("p (k b) -> p k b", k=NJ)[:, :, 0:bgrp])
            for g in (G_F, G_O):
                for q in range(NQ):
                    gs = slice(g * H, (g + 1) * H)
                    nc.sync.dma_start(w_sb[q][:, :, gs], w8_ap[q][:, :, gs])
            nc.sync.dma_start(
                hT[1][:, :, 0:bgrp],
                hT_in.ap()[1].rearrange("p (k b) -> p k b", k=NJ)[:, :, 0:bgrp])
            for t in range(T):
                ap_t = hT_in.ap()[t].rearrange("p (k b) -> p k b", k=NJ)
                if t < 2:
                    nc.sync.dma_start(hT[t][:, :, bgrp:b_loc],
                                      ap_t[:, :, bgrp:b_loc])
                else:
                    nc.sync.dma_start(hT[t][:], ap_t)
            vd_sb = const.tile([P, NJ], bf16, name="vd_sb")
            nc.sync.dma_start(vd_sb[:], vd_in.ap())
            vh_sb = const.tile([P, CH], bf16, name="vh_sb")
            nc.sync.dma_start(vh_sb[:], vh_in.ap())
            h9_t = []
            for bt in range(NBT):
                h9 = const.tile([P, CH], bf16, name=f"h9_{bt}",
                                tag=f"h9_{bt}")
                nc.sync.dma_start(h9[:], h9_in.ap()[bt * P:(bt + 1) * P, :])
                h9_t.append(h9)
            yh = const.tile([P, NBT], f32, name="yh")

            def k2_matmuls(ps, gate, t, bg):
                """K=2 bias/y matmuls: 4-up tile_position pack, start=True
                clears the 4 banks (each j-region is exactly one bank)."""
                for j in range(NJ):
                    mi = gate * NJ + j
                    if y8:
                        nc.tensor.matmul(
                            ps[:, j * bgrp:(j + 1) * bgrp],
                            wb_sb[32 * j:32 * j + 1, :,
                                  mi * P:(mi + 1) * P],
                            yt_sb[32 * j:32 * j + 1, :,
                                  t * b_loc + bg * bgrp:
                                  t * b_loc + (bg + 1) * bgrp],
                            start=True, stop=False,
                            tile_position=(32 * j, 0),
                            perf_mode=DR,
                            skip_group_check=True)
                    else:
                        nc.tensor.matmul(
                            ps[:, j * bgrp:(j + 1) * bgrp],
                            wb_sb[32 * j:32 * j + 2, mi * P:(mi + 1) * P],
                            yt_sb[32 * j:32 * j + 2,
                                  t * b_loc + bg * bgrp:
                                  t * b_loc + (bg + 1) * bgrp],
                            start=True, stop=False,
                            tile_position=(32 * j, 0),
                            skip_group_check=True)

            def dr_matmuls(ps, gate, t, bg):
                """fp8 DoubleRow matmuls, K=256 each (two 128-ch regions)."""
                for j in range(NJ):
                    mi = gate * NJ + j
                    for q in range(NQ):
                        nc.tensor.matmul(
                            ps[:, j * bgrp:(j + 1) * bgrp],
                            w_sb[q][:, :, mi * P:(mi + 1) * P],
                            hT[t][:, 2 * q:2 * q + 2,
                                  bg * bgrp:(bg + 1) * bgrp],
                            start=False, stop=(q == NQ - 1),
                            perf_mode=DR,
                            skip_group_check=True)

            def gate_matmul(gate, t, bg):
                ps = psum.tile([P, GW], f32, name="ps_big", tag="ps")
                k2_matmuls(ps, gate, t, bg)
                dr_matmuls(ps, gate, t, bg)
                return ps

            # ---- main loop ----
            # Per bg: 30 gate tiles in order [I0 G0 I1 G1 F1 ... I9 G9 F9 O9]
            # processed in PAIRS: both tiles' K=2 groups issue together, then
            # both DR batches — halving the PE tile-config switch cost.
            y_d_t = []
            c_bg = []
            so_bg = []
            for bg in range(NBG):
                c_t = const.tile([P, GW], bf16 if c_bf16 else f32,
                                 name=f"c_{bg}", tag=f"c{bg}")
                so_t = fin.tile([P, GW], bf16, name="so", tag=f"so{bg}",
                                bufs=1)
                so_bg.append(so_t)

                gate_list = [(G_I, 0), (G_G, 0)]
                for t in range(1, T):
                    gate_list += [(G_I, t), (G_G, t), (G_F, t)]
                gate_list.append((G_O, T - 1))

                tiles = {}      # (gate, t) -> work tile with ACT output

                def emit_consumers(gate, t, c_t=c_t, tiles=tiles):
                    """DVE ops that become ready once (gate, t) is activated.

                    Called right after the ACT emission for (gate, t); FIFO
                    order on DVE preserves the c-chain sequence."""
                    if gate == G_G:
                        si = tiles.pop((G_I, t))
                        tg = tiles.pop((G_G, t))
                        if t == 0:
                            nc.vector.tensor_tensor(c_t[:], si[:], tg[:],
                                                    ALU.mult)
                        else:
                            m = work.tile([P, GW], bf16, name="m", tag="m")
                            nc.vector.tensor_tensor(m[:], si[:], tg[:],
                                                    ALU.mult)
                            tiles[("m", t)] = m
                    elif gate == G_F:
                        sf = tiles.pop((G_F, t))
                        m = tiles.pop(("m", t))
                        nc.vector.tensor_tensor(c_t[:], c_t[:], sf[:],
                                                ALU.mult)
                        nc.vector.tensor_tensor(c_t[:], c_t[:], m[:],
                                                ALU.add)

                if pair_gates:
                    for k in range(0, len(gate_list), 2):
                        pair = gate_list[k:k + 2]
                        pss = []
                        for gate, t in pair:
                            ps = psum.tile([P, GW], f32, name="ps_big",
                                           tag="ps")
                            k2_matmuls(ps, gate, t, bg)
                            pss.append(ps)
                        for (gate, t), ps in zip(pair, pss):
                            dr_matmuls(ps, gate, t, bg)
                        for (gate, t), ps in zip(pair, pss):
                            if gate == G_O:
                                nc.scalar.activation(so_t[:], ps[:],
                                                     AF.Sigmoid, scale=INV)
                            else:
                                func = AF.Tanh if gate == G_G else AF.Sigmoid
                                wt = work.tile([P, GW], bf16, name="act",
                                               tag="si" if gate == G_I else
                                               ("tg" if gate == G_G else
                                                "sf"))
                                nc.scalar.activation(wt[:], ps[:], func,
                                                     scale=INV)
                                tiles[(gate, t)] = wt
                            emit_consumers(gate, t)
                else:
                    for gate, t in gate_list:
                        ps = gate_matmul(gate, t, bg)
                        if gate == G_O:
                            # per-j chunks: lets the finalize chain start
                            # ~1.3us earlier at the tail
                            for j in range(NJ):
                                sl = slice(j * bgrp, (j + 1) * bgrp)
                                nc.scalar.activation(so_t[:, sl], ps[:, sl],
                                                     AF.Sigmoid, scale=INV)
                        else:
                            func = AF.Tanh if gate == G_G else AF.Sigmoid
                            wt = work.tile([P, GW], bf16, name="act",
                                           tag="si" if gate == G_I else
                                           ("tg" if gate == G_G else "sf"))
                            nc.scalar.activation(wt[:], ps[:], func,
                                                 scale=INV)
                            tiles[(gate, t)] = wt
                        emit_consumers(gate, t)

                c_bg.append(c_t)
                # y_h = h9 . v_h: all 8 batch tiles emitted after bg0's loop
                # (DVE has slack while bg1 computes; outh DMA leaves the
                # tail)
                if bg == 0:
                    for bt in range(NBT):
                        tmp = work.tile([P, CH], bf16, name="tmp9",
                                        tag="tmp9")
                        nc.vector.tensor_tensor(tmp[:], h9_t[bt][:],
                                                vh_sb[:], ALU.mult)
                        nc.vector.tensor_reduce(yh[:, bt:bt + 1], tmp[:],
                                                mybir.AxisListType.X,
                                                ALU.add)
                    nc.sync.dma_start(outh.ap(), yh[:])

                # d = sigma(o) * tanh(c) per j-chunk now (ACT/DVE only, no
                # PSUM — bg0's part overlaps bg1's main loop); dots deferred
                d_bg = []
                for j in range(NJ):
                    sl = slice(j * bgrp, (j + 1) * bgrp)
                    tc9 = fin.tile([P, bgrp], bf16, name="tc9", tag="tc9",
                                   bufs=3)
                    nc.scalar.activation(tc9[:], c_t[:, sl], AF.Tanh)
                    d = fin.tile([P, bgrp], bf16, name="d", tag=f"d{bg}_{j}",
                                 bufs=1)
                    nc.vector.tensor_tensor(d[:], so_t[:, sl], tc9[:],
                                            ALU.mult)
                    d_bg.append(d)
                y_d_t.append(d_bg)

            # ---- final dots (after all gate psum tiles -> no rotation
            # stalls) ----
            for bg in range(NBG):
                ps_y = psum.tile([1, bgrp], f32, name="ps_y", tag="ps")
                for j in range(NJ):
                    nc.tensor.matmul(ps_y[:], vd_sb[:, j:j + 1],
                                     y_d_t[bg][j][:],
                                     start=(j == 0), stop=(j == NJ - 1))
                y_d = fin.tile([1, bgrp], f32, name="y_d", tag=f"y_d{bg}",
                               bufs=1)
                nc.vector.tensor_copy(y_d[:], ps_y[:])
                nc.sync.dma_start(outd.ap()[bg * bgrp:(bg + 1) * bgrp],
                                  y_d[:])


    nc.compile()
    return nc


def _host_prep(inputs):
    W_hh = np.asarray(inputs["W_hh"], np.float32)
    W_ih = np.asarray(inputs["W_ih"], np.float32)
    b = (np.asarray(inputs["b_ih"], np.float32)
         + np.asarray(inputs["b_hh"], np.float32))          # [2048]
    fc1_w = np.asarray(inputs["fc1_w"], np.float32)
    fc2_w = np.asarray(inputs["fc2_w"], np.float32)
    v = (fc2_w @ fc1_w)[0]                                   # [1024]
    c0 = float(np.asarray(inputs["fc1_b"], np.float32) @ fc2_w[0]
               + np.asarray(inputs["fc2_b"], np.float32)[0])

    NJ = H // P
    # w8[q, p, i, m] = fp8(W_hh[m, 128*(2q+i) + p] * S)
    W8T = (W_hh * S).astype(F8).T                            # [512, 2048]
    w8 = np.ascontiguousarray(
        W8T.reshape(2, 2, P, 4 * H).transpose(0, 2, 1, 3)
    ).reshape(2, P, 2 * 4 * H)
    wb_f32 = np.stack([W_ih[:, 0] * S, b * S])               # [2, 2048]
    if Y8:
        wih_b = np.ascontiguousarray(wb_f32.astype(F8))[None]
    else:
        wih_b = np.ascontiguousarray(wb_f32.astype(BF16))
    v_d = np.ascontiguousarray(v[:H].reshape(NJ, P).T.copy())       # [128,4]
    v_h = np.ascontiguousarray(np.tile(v[H:][None, :], (P, 1)))     # [128,512]
    return w8, wih_b, v_d.astype(BF16), v_h.astype(BF16), c0


def _install_ntff_shim():
    """Best-effort: recreate antenv.axon_hooks so trace=True can profile."""
    import sys as _sys
    import types as _types
    try:
        import antenv.axon_hooks  # noqa: F401
        return
    except ImportError:
        pass
    try:
        import antenv
        from trn_agent_boot.trn_boot import _ntff_profile_via_ctypes
        hook = _ntff_profile_via_ctypes("/opt/axon/libaxon_pjrt.so")
        mod = _types.ModuleType("antenv.axon_hooks")
        _state = {"hook": hook}
        mod.set_axon_ntff_profile_hook = lambda hk: _state.__setitem__("hook", hk)
        mod.get_axon_ntff_profile_hook = lambda: _state["hook"]
        _sys.modules["antenv.axon_hooks"] = mod
        antenv.axon_hooks = mod
    except Exception:
        pass


def make_in_maps(inputs):
    w8, wih_b, v_d, v_h, c0 = _host_prep(inputs)
    h = np.asarray(inputs["h"], np.float32)
    y = np.asarray(inputs["y_seq"], np.float32)
    NJ = H // P
    in_maps = []
    for c in range(N_CORES):
        sl = slice(c * B_LOC, (c + 1) * B_LOC)
        h_sl = h[sl]                                         # [1024, 10, 512]
        h8 = h_sl.astype(F8)
        # hT8[t, p, k*1024 + b] = fp8(h[b, t, 128k + p])
        hT8 = np.ascontiguousarray(
            h8.transpose(1, 2, 0)                            # [T, CH, b_loc]
            .reshape(T, NJ, P, B_LOC)
            .transpose(0, 2, 1, 3)                           # [T, P, NJ, b]
        ).reshape(T, P, NJ * B_LOC)
        if Y8:
            yt = np.empty((1, 2, T * B_LOC), F8)
            yt[0, 0] = y[sl].T.astype(F8).reshape(-1)
            yt[0, 1] = np.ones(T * B_LOC, F8)
            wb_key = "wb8"
        else:
            yt = np.empty((2, T, B_LOC), BF16)
            yt[0] = y[sl].T.astype(BF16)
            yt[1] = np.ones((T, B_LOC), BF16)
            wb_key = "wih_b"
        in_maps.append({
            "hT8": hT8,
            "h9": np.ascontiguousarray(h_sl[:, T - 1, :].astype(BF16)),
            "yt_aug": yt,
            "w8": w8, wb_key: wih_b,
            "v_d": v_d, "v_h": v_h,
        })
    return in_maps, c0


def run(inputs, trace=False, retries=2):
    key = "full"
    if key not in _compiled:
        _compiled[key] = build_nc()
    nc = _compiled[key]

    if trace:
        _install_ntff_shim()

    in_maps, c0 = make_in_maps(inputs)
    res = None
    for attempt in range(retries + 1):
        try:
            res = run_bass_kernel_spmd(nc, in_maps,
                                       core_ids=list(range(N_CORES)),
                                       trace=trace)
            break
        except Exception:
            # the device sporadically wedges (NRT_EXEC_UNIT_UNRECOVERABLE);
            # a simple retry after poking the runtime usually recovers it
            if attempt == retries:
                raise
            import time as _time
            _time.sleep(2.0)
    outs = []
    for c in range(N_CORES):
        r = res.results[c]
        y_core = (r["out_d"] + r["out_h"].T.reshape(-1) + c0)
        outs.append(y_core.astype(np.float32))
    return np.concatenate(outs)[:, None], res


def kernel(**inputs):
    out, _ = run(inputs, trace=False)
    return out
